# revision 46
# baseline (speedup 1.0000x reference)
"""Causal multi-head attention with (buggy-faithful) RoPE on 8 trn2 cores.

Problem: B=4, S=2048, D=1024, H=16 heads of dim 64, fp32.
Sharding: core c handles batch b=c//2 and head-group g=c%2 (8 heads).
Each core computes partial_out = attn(x_b, heads g) @ wo[rows g]; the host
sums the two partials per batch and adds the bias.

v2: fp8 (e4m3) DoubleRow matmuls for the PE-heavy stages. Double FP8 packs
two contraction rows per partition (K_eff=256) for 2x tensor-engine
throughput. Precision plan (validated host-side, rel err ~2e-3):
- Q/K projections: fp8 DoubleRow (q/k noise ~4% is washed out by softmax).
- V projection: bf16 for kv<512, fp8 DoubleRow for kv>=512.
- P@V + the attention probs pt: fp8 for q>=512 (t>=1), bf16 for t=0. Early
  rows have peaked attention (y ~= v_k), so V/pt quantization error there
  lands directly in the output max; rows>=512 average over >=512 positions.
- Output projection: fp8 DoubleRow for q>=512, bf16 for t=0.
- Scores stay bf16 (K=64 per head; two heads pack PE row-groups 0-1/2-3
  concurrently via tile_position, so bf16 scores already run at full rate).

Scale folding keeps every fp8 tensor in e4m3's happy range (max 240):
wq/wk *128 (host) with /16 folded into the bf16 sin table -> qt,kt = 8*q,8*k
(exp scale 0.125/64); wv *128 with *1/8 in the drain -> v16 = 16*v; ones
column 1.0 -> yt = 16*y; wo *8 -> out psum = 128*out, *1/128 in the drain.

On-device layout (per core):
- x8 [4][128, 2, S] fp8: feature d = 256c+128i+p (DoubleRow pair on i).
- QT/KT [512, 2048] bf16 feature-major; V as [128, 2, 8, 65] fp8 per 256-kv
  chunk (65th col = ones -> softmax denominator accumulates in the P@V
  matmul) + bf16 [128, 8, 65] for kv<512.
- S^T [kv, q] per head; exp needs no max subtraction (|scores/8| < ~3).
- pt8 [128, 2, 2, 512] fp8 per 256-kv chunk: (kv-half i, parity e, q).
- P@V full 256-chunks: one DoubleRow matmul per head; diagonal 128-chunks:
  plain fp8 matmuls over the trapezoid, mask boundary via affine_select.
"""

import numpy as np
import ml_dtypes

import concourse.bacc as bacc
import concourse.mybir as mybir
import concourse.tile as tile
from concourse.bass_utils import run_bass_kernel_spmd

B, S, D = 4, 2048, 1024
H = 16
AOD = 64
HL = 8            # heads per core
FL = HL * AOD     # 512 local features
NCORES = 8
NCH8 = 4          # fp8 DoubleRow contraction chunks (256 features each)
NSQ = 4           # sq tiles of 512
NP = FL // 128    # 4 feature chunks (head pairs)
NT16 = S // 128   # 16 seq chunks of 128

F32 = mybir.dt.float32
BF16 = mybir.dt.bfloat16
FP8 = mybir.dt.float8e4
EXP = mybir.ActivationFunctionType.Exp
DR = mybir.MatmulPerfMode.DoubleRow
EXP_SCALE = 0.125 / 64.0   # qt,kt carry x8 each

_CACHED = {}


def _alu():
    from concourse.alu_op_type import AluOpType
    return AluOpType


def _emit_vproj(nc, P, sb, q, rep):
    """V projection for seq tile q (128 positions): writes v8 (fp8, 16*v)
    and, for q<4, also vb (bf16, 16*v) for the t=0 path."""
    (p_x, p_xb, p_w, p_wb, p_wo, p_wob, p_qk, p_v, p_vb, p_sin, p_pt,
     p_ptb, p_yt, p_ytb, p_r, p_os, ps_proj, ps_s, ps_o) = P
    ps = ps_proj.tile([128, FL], F32, tag="psp", name=f"r{rep}vps{q}")
    if q < 4:
        for c in range(8):
            nc.tensor.matmul(
                ps[:], sb["xb"][c][:, 128 * q:128 * (q + 1)], sb["wvb"][c][:],
                start=(c == 0), stop=(c == 7))
    else:
        for c in range(NCH8):
            nc.tensor.matmul(
                ps[:], sb["x8"][c][:, :, 128 * q:128 * (q + 1)],
                sb["wv8"][c][:], perf_mode=DR,
                start=(c == 0), stop=(c == NCH8 - 1))
    nc.vector.tensor_scalar_mul(
        sb["v8"][q // 2][:, q % 2, :, 0:64],
        ps[:].rearrange("p (h d) -> p h d", h=HL), 0.125)
    if q < 4:
        nc.vector.tensor_scalar_mul(
            sb["vb"][q][:, :, 0:64],
            ps[:].rearrange("p (h d) -> p h d", h=HL), 0.125)


def _emit_body(nc, P, dram, rep):
    """One full forward pass for this core's shard."""
    mult = _alu().mult
    is_ge = _alu().is_ge
    (p_x, p_xb, p_w, p_wb, p_wo, p_wob, p_qk, p_v, p_vb, p_sin, p_pt,
     p_ptb, p_yt, p_ytb, p_r, p_os, ps_proj, ps_s, ps_o) = P
    R = f"r{rep}"

    # ---- resident loads, split across the qSP and qPool DMA queues so the
    # first-needed tensors land in ~3us instead of ~17us serial. The Pool
    # queue only carries the early small loads: gpsimd DMA blocks the Pool
    # engine for the transfer, and Pool must stay free for the causal-mask
    # affine_selects in the exp->PV chain. ----
    sb = {}
    # warm the ACT exp table during the DMA window (saves ~1.4us at first
    # exp); the table persists, so only the first rep needs it
    if rep == 0:
        warm = p_r.tile([1, 16], F32, tag="warm", name=f"{R}warm")
        nc.gpsimd.memset(warm[:], 0.0)
        nc.scalar.activation(warm[:], warm[:], EXP, scale=1.0)
    sb["wq8"] = []
    sb["wk8"] = []
    for c in range(NCH8):
        t = p_w.tile([128, 2, FL], FP8, tag="w", name=f"{R}wq8{c}")
        (nc.sync if c < 2 else nc.gpsimd).dma_start(t[:], dram["wq8"][c])
        sb["wq8"].append(t)
    sb["x8"] = [p_x.tile([128, 2, S], FP8, tag="x", name=f"{R}x{c}")
                for c in range(NCH8)]
    for c in range(NCH8):   # t=0 column slice first: compute starts early
        nc.sync.dma_start(
            sb["x8"][c][:, :, 0:512], dram["x8"][c, :, :, 0:512])
    for c in range(NCH8):
        t = p_w.tile([128, 2, FL], FP8, tag="w", name=f"{R}wk8{c}")
        nc.gpsimd.dma_start(t[:], dram["wk8"][c])
        sb["wk8"].append(t)
    sin_sb = p_sin.tile([128, S], BF16, tag="sin", name=f"{R}sin_sb")
    nc.gpsimd.dma_start(sin_sb[:], dram["sinq"])
    for c in range(2):
        nc.sync.dma_start(
            sb["x8"][c][:, :, 512:1024], dram["x8"][c, :, :, 512:1024])
    sb["xb"] = []
    for c in range(8):
        t = p_xb.tile([128, 512], BF16, tag="xb", name=f"{R}xb{c}")
        (nc.sync if c < 4 else nc.gpsimd).dma_start(t[:], dram["xb"][c])
        sb["xb"].append(t)
    sb["wvb"] = []
    for c in range(8):
        t = p_wb.tile([128, FL], BF16, tag="wb", name=f"{R}wvb{c}")
        (nc.sync if c < 4 else nc.gpsimd).dma_start(t[:], dram["wvb"][c])
        sb["wvb"].append(t)
    for c in range(2, NCH8):
        nc.gpsimd.dma_start(
            sb["x8"][c][:, :, 512:1024], dram["x8"][c, :, :, 512:1024])
    for c in range(NCH8):
        (nc.sync if c < 2 else nc.gpsimd).dma_start(
            sb["x8"][c][:, :, 1024:1536], dram["x8"][c, :, :, 1024:1536])
    sb["wv8"] = []
    for c in range(NCH8):
        t = p_w.tile([128, 2, FL], FP8, tag="w", name=f"{R}wv8{c}")
        nc.sync.dma_start(t[:], dram["wv8"][c])
        sb["wv8"].append(t)
    for c in range(NCH8):
        nc.sync.dma_start(
            sb["x8"][c][:, :, 1536:2048], dram["x8"][c, :, :, 1536:2048])
    sb["wo8"] = []
    for c in range(2):
        t = p_wo.tile([128, 2, D], FP8, tag="wo", name=f"{R}wo8{c}")
        nc.sync.dma_start(t[:], dram["wo8"][c])
        sb["wo8"].append(t)
    sb["wob"] = []
    for c in range(NP):
        t = p_wob.tile([128, D], BF16, tag="wob", name=f"{R}wob{c}")
        nc.sync.dma_start(t[:], dram["wob"][c])
        sb["wob"].append(t)

    # V tiles: fp8 per 256-kv chunk (+ bf16 for kv<512); ones col up front
    sb["v8"] = []
    for c in range(8):
        # 66-wide per head: DoubleRow LDWEIGHTS needs the pair-dim byte
        # step (8*66=528) to be 16-aligned; col 64 = ones, col 65 = pad
        vt = p_v.tile([128, 2, HL, 66], FP8, tag="v", name=f"{R}v8_{c}")
        nc.gpsimd.memset(vt[:, :, :, 64:65], 1.0)
        sb["v8"].append(vt)
    sb["vb"] = []
    for c in range(4):
        vt = p_vb.tile([128, HL, 65], BF16, tag="vb", name=f"{R}vb{c}")
        nc.gpsimd.memset(vt[:, :, 64:65], 1.0)
        sb["vb"].append(vt)

    # yt: bf16 [128, 512] per pair for t=0; fp8 DoubleRow layout for t>=1
    sb["ytb"] = [p_ytb.tile([128, 512], BF16, tag="ytb", name=f"{R}ytb{i}")
                 for i in range(NP)]
    sb["yt8"] = [p_yt.tile([128, 2, 1536], FP8, tag="yt", name=f"{R}yt8{i}")
                 for i in range(2)]
    qt_sb = [p_qk.tile([128, S], BF16, tag="qk", name=f"{R}qt{i}")
             for i in range(NP)]
    kt_sb = [p_qk.tile([128, S], BF16, tag="qk", name=f"{R}kt{i}")
             for i in range(NP)]

    def emit_qkproj(p, t):
        # Q and K projection slice (fp8 DoubleRow), rope fused into the drain
        for w_sb, dst, nm in ((sb["wq8"], qt_sb[p], "q"),
                              (sb["wk8"], kt_sb[p], "k")):
            ps = ps_proj.tile([128, 512], F32, tag="psp",
                              name=f"{R}{nm}ps{p}{t}")
            for c in range(NCH8):
                nc.tensor.matmul(
                    ps[:],
                    w_sb[c][:, :, 128 * p:128 * (p + 1)],
                    sb["x8"][c][:, :, 512 * t:512 * (t + 1)],
                    perf_mode=DR,
                    start=(c == 0), stop=(c == NCH8 - 1))
            if p == 0 and t == 0 and nm == "k":
                # cold start: the first score chunk only reads kt[:, 0:128];
                # a split drain lets it go ~0.5us earlier
                nc.vector.tensor_tensor(
                    out=dst[:, 0:128], in0=ps[:, 0:128],
                    in1=sin_sb[:, 0:128], op=mult)
                nc.vector.tensor_tensor(
                    out=dst[:, 128:512], in0=ps[:, 128:512],
                    in1=sin_sb[:, 128:512], op=mult)
            else:
                nc.vector.tensor_tensor(
                    out=dst[:, 512 * t:512 * (t + 1)],
                    in0=ps[:], in1=sin_sb[:, 512 * t:512 * (t + 1)],
                    op=mult)

    emit_qkproj(0, 0)
    # attention for heads 2p (e=0) and 2p+1 (e=1). Scores for both parities
    # share one [128, 2, 512] S^T psum tile; their K=64 matmuls pack PE
    # row-groups 0-1/2-3 concurrently, and one exp covers both parities.
    # P@V runs one chunk behind the S matmuls so PE never waits on ACT.
    # Block order: pairs {0,1} interleaved over t first — two pairs' early
    # exp work covers the DMA window for the later x slices — then p-outer
    # for {2,3} (long blocks keep the psum-pool recycling off the exp path).
    SEQ = ([(0, 0), (0, 1), (1, 0), (1, 1), (0, 2), (1, 2), (0, 3), (1, 3)]
           + [(p, t) for p in (2, 3) for t in range(NSQ)])

    def emit_pv0(pp, o_ps_l, pts_l):
        for cp, offp, ptp in pts_l:
            for e in range(2):
                nc.tensor.matmul(
                    o_ps_l[e][:, offp:512], sb["vb"][cp][:, 2 * pp + e, :],
                    ptp[:, e, offp:512],
                    start=(cp == 0), stop=(cp == 3))

    def emit_norm(pp, tt, o_ps_l):
        # recip of the ones-row, broadcast, scale the 64 rows. Last tile
        # runs in 256-wide halves so the tail O-proj starts as soon as the
        # first half of every pair's yt lands.
        halves = ((0, 256), (256, 512)) if tt == NSQ - 1 else ((0, 512),)
        for h0, h1 in halves:
            for e in range(2):
                recip = p_r.tile([1, h1 - h0], F32, tag="rc",
                                 name=f"{R}rc{pp}_{tt}{e}_{h0}")
                nc.vector.reciprocal(recip[:], o_ps_l[e][64:65, h0:h1])
                rb = p_r.tile([64, h1 - h0], F32, tag="rb",
                              name=f"{R}rb{pp}_{tt}{e}_{h0}")
                nc.gpsimd.partition_broadcast(rb[:], recip[:], channels=64)
                if tt == 0:
                    dst = sb["ytb"][pp][64 * e:64 * (e + 1), h0:h1]
                else:
                    dst = sb["yt8"][pp // 2][64 * e:64 * (e + 1), pp % 2,
                                             512 * tt - 512 + h0:
                                             512 * tt - 512 + h1]
                nc.vector.tensor_tensor(
                    out=dst, in0=o_ps_l[e][0:64, h0:h1], in1=rb[:], op=mult)

    def emit_oproj_unit(q, o, tt):
        # tail units alternate through the pso pool (idle after the last
        # norm) so psp frees early for the next rep's first projections
        pool = ps_o if (tt == NSQ - 1 and o == 1) else ps_proj
        ps = pool.tile([128, 512], F32, tag="psp" if pool is ps_proj else "pso",
                       name=f"{R}ops{q}{o}")
        if tt == 0:
            for pp in range(NP):
                nc.tensor.matmul(
                    ps[:],
                    sb["ytb"][pp][:, 128 * q:128 * (q + 1)],
                    sb["wob"][pp][:, 512 * o:512 * (o + 1)],
                    start=(pp == 0), stop=(pp == NP - 1))
        else:
            for c2 in range(2):
                nc.tensor.matmul(
                    ps[:],
                    sb["yt8"][c2][:, :, 128 * q - 512:128 * (q + 1) - 512],
                    sb["wo8"][c2][:, :, 512 * o:512 * (o + 1)],
                    perf_mode=DR, start=(c2 == 0), stop=(c2 == 1))
        os_t = p_os.tile([128, 512], F32, tag="os", name=f"{R}os{q}{o}")
        if tt == NSQ - 1:
            # tail: ACT idles after the last exp and DVE after the last
            # yt; split the drains. Pool's DMA queue has no selects left.
            if o == 0:
                nc.scalar.mul(os_t[:], ps[:], 1.0 / 128.0)
            else:
                nc.vector.tensor_scalar_mul(os_t[:], ps[:], 1.0 / 128.0)
            eng = nc.sync if o == 0 else nc.gpsimd
        else:
            nc.vector.tensor_scalar_mul(os_t[:], ps[:], 1.0 / 128.0)
            eng = nc.sync
        eng.dma_start(
            dram["out"][128 * q:128 * (q + 1), 512 * o:512 * (o + 1)],
            os_t[:])

    def emit_pv8_flush(pp, tt, o_ps_l, ptt, pvq):
        n = len(pvq)
        for idx, (kind, cc2, _) in enumerate(pvq):
            first, last = idx == 0, idx == n - 1
            if kind == "full":
                ptp = ptt[("f", cc2)]
                for e in range(2):
                    nc.tensor.matmul(
                        o_ps_l[e][:, :], sb["v8"][cc2][:, :, 2 * pp + e, 0:65],
                        ptp[:, :, e, :], perf_mode=DR, start=first, stop=last)
            else:
                c = cc2
                off = 128 * (c - 4 * tt)
                ptp = ptt[("f", c // 2)]
                for e in range(2):
                    nc.tensor.matmul(
                        o_ps_l[e][:, off:512],
                        sb["v8"][c // 2][:, c % 2, 2 * pp + e, 0:65],
                        ptp[:, c % 2, e, off:512], start=first, stop=last)

    hold00 = []
    hold01 = []
    oproj_pending = []
    for bi, (p, t) in enumerate(SEQ):
        if True:
            nxt = SEQ[bi + 1] if bi + 1 < len(SEQ) else None
            o_ps = [ps_o.tile([65, 512], F32, tag="pso",
                              name=f"{R}o{p}_{t}_{i}") for i in range(2)]

            if t == 0:
                # bf16 path (4 diagonal 128-chunks). vproj/PV emitted after
                # the score/exp stream: their xb/wvb DMAs land late and must
                # not clog the PE wait queue ahead of the scores. Pair 0's
                # PV+norm are deferred into pair 1's block for the same
                # reason.
                pts = []
                for c in range(4):
                    off = 128 * c
                    w = 512 - off
                    s_ps = ps_s.tile([128, 2, 512], F32, tag="s",
                                     name=f"{R}s{p}_0_{c}")
                    for e in range(2):
                        nc.tensor.matmul(
                            s_ps[:, e, 0:w],
                            kt_sb[p][64 * e:64 * (e + 1),
                                     128 * c:128 * (c + 1)],
                            qt_sb[p][64 * e:64 * (e + 1), off:512],
                            start=True, stop=True)
                    if c == 1 and nxt is not None:
                        emit_qkproj(*nxt)
                    pt = p_ptb.tile([128, 2, 512], BF16, tag="ptb",
                                    name=f"{R}pt{p}_0_{c}")
                    if off:
                        nc.scalar.activation(
                            pt[:, :, off:512], s_ps[:, :, 0:w],
                            EXP, scale=EXP_SCALE)
                    else:
                        nc.scalar.activation(
                            pt[:].rearrange("p a b -> p (a b)"),
                            s_ps[:].rearrange("p a b -> p (a b)"),
                            EXP, scale=EXP_SCALE)
                    nc.gpsimd.affine_select(
                        out=pt[:, :, off:off + 128],
                        in_=pt[:, :, off:off + 128],
                        compare_op=is_ge, fill=0.0, base=0,
                        pattern=[[0, 2], [1, 128]], channel_multiplier=-1)
                    pts.append((c, off, pt))
                if p == 0:
                    hold00.append((o_ps, pts))
                    continue
                if p == 1 and hold00:
                    # the full deferred train: vb/v8 projections (late DMA)
                    # plus pair 0's t0 and t1 PV+norm, all emitted behind
                    # this block's exp stream so no block's scores ever sit
                    # behind it in the PE queue
                    for q in range(8):
                        _emit_vproj(nc, P, sb, q, rep)
                    o_ps00, pts00 = hold00.pop()
                    emit_pv0(0, o_ps00, pts00)
                    emit_norm(0, 0, o_ps00)
                    o_ps01, ptt01, pvq01 = hold01.pop()
                    emit_pv8_flush(0, 1, o_ps01, ptt01, pvq01)
                    emit_norm(0, 1, o_ps01)
                emit_pv0(p, o_ps, pts)
            else:
                # fp8 path: full 256-chunks DoubleRow, diagonal 128-chunks
                # plain fp8. pt8 per 256-chunk: [128, 2(kv-half), 2(par), 512].
                pt_tiles = {}
                pv_queue = []   # (kind, c2_or_c, gate_cc)
                for c2 in range(2 * t):
                    pv_queue.append(("full", c2, 2 * c2 + 1))
                for c in range(4 * t, 4 * (t + 1)):
                    pv_queue.append(("diag", c, c))
                n_pv = len(pv_queue)
                emitted = [0]

                def emit_pv8(stop_at):
                    while emitted[0] < stop_at:
                        kind, cc2, _ = pv_queue[emitted[0]]
                        first = emitted[0] == 0
                        last = emitted[0] == n_pv - 1
                        if kind == "full":
                            ptp = pt_tiles[("f", cc2)]
                            for e in range(2):
                                nc.tensor.matmul(
                                    o_ps[e][:, :],
                                    sb["v8"][cc2][:, :, 2 * p + e, 0:65],
                                    ptp[:, :, e, :], perf_mode=DR,
                                    start=first, stop=last)
                        else:
                            c = cc2
                            off = 128 * (c - 4 * t)
                            ptp = pt_tiles[("f", c // 2)]
                            for e in range(2):
                                nc.tensor.matmul(
                                    o_ps[e][:, off:512],
                                    sb["v8"][c // 2][:, c % 2, 2 * p + e, 0:65],
                                    ptp[:, c % 2, e, off:512],
                                    start=first, stop=last)
                        emitted[0] += 1

                for cc in range(4 * (t + 1)):
                    c2, i = cc // 2, cc % 2
                    dc = cc - 4 * t
                    off = 128 * dc if dc > 0 else 0
                    w = 512 - off
                    s_ps = ps_s.tile([128, 2, 512], F32, tag="s",
                                     name=f"{R}s{p}_{t}_{cc}")
                    for e in range(2):
                        nc.tensor.matmul(
                            s_ps[:, e, 0:w],
                            kt_sb[p][64 * e:64 * (e + 1),
                                     128 * cc:128 * (cc + 1)],
                            qt_sb[p][64 * e:64 * (e + 1),
                                     512 * t + off:512 * (t + 1)],
                            start=True, stop=True)
                    if p == 0 and cc < 4 and bi != 1:
                        _emit_vproj(nc, P, sb, 4 * t + cc, rep)
                    if cc == 1 and nxt is not None:
                        emit_qkproj(*nxt)
                    if ("f", c2) not in pt_tiles:
                        pt_tiles[("f", c2)] = p_pt.tile(
                            [128, 2, 2, 512], FP8, tag="pt",
                            name=f"{R}pt{p}_{t}_{c2}")
                    pt = pt_tiles[("f", c2)]
                    if off:
                        nc.scalar.activation(
                            pt[:, i, :, off:512], s_ps[:, :, 0:w],
                            EXP, scale=EXP_SCALE)
                    else:
                        nc.scalar.activation(
                            pt[:, i, :, :].rearrange("p a b -> p (a b)"),
                            s_ps[:].rearrange("p a b -> p (a b)"),
                            EXP, scale=EXP_SCALE)
                    if dc >= 0:
                        # causal boundary within [off:off+128]
                        nc.gpsimd.affine_select(
                            out=pt[:, i, :, off:off + 128],
                            in_=pt[:, i, :, off:off + 128],
                            compare_op=is_ge, fill=0.0, base=0,
                            pattern=[[0, 2], [1, 128]], channel_multiplier=-1)
                    # emit PV units whose pt is complete, one chunk behind
                    if bi != 1:
                        ready = sum(1 for u in pv_queue if u[2] <= cc - 1)
                        emit_pv8(ready)
                    if oproj_pending and (oproj_pending[0][2] > 0
                                          or cc % 2 == 1):
                        # t0-sourced units cost ~0.9us of PE (4 bf16
                        # matmuls) — more than one exp-chunk cadence, so
                        # only every other chunk; DR units are cheap
                        emit_oproj_unit(*oproj_pending.pop(0))
                if bi == 1:
                    # stash: this block's PV depends on the vproj train
                    # (late DMA); both are deferred into block (1,0)
                    hold01.append((o_ps, pt_tiles, pv_queue))
                else:
                    emit_pv8(n_pv)

            if bi == 1:
                continue
            emit_norm(p, t, o_ps)

            if p == NP - 1:
                # output projection for the seq tiles this t completes.
                # t<3 units are queued and interleaved into the NEXT block's
                # chunk loop so their matmuls/drains don't sit ahead of that
                # block's scores in the PE/DVE streams; t=3 is the tail.
                units = [(q, o, t) for q in range(4 * t, 4 * (t + 1))
                         for o in range(2)]
                if t < NSQ - 1:
                    oproj_pending.extend(units)
                else:
                    while oproj_pending:
                        emit_oproj_unit(*oproj_pending.pop(0))
                    for u in units:
                        emit_oproj_unit(*u)


def build_nc(reps=1):
    key = ("nc", reps)
    if key in _CACHED:
        return _CACHED[key]
    from contextlib import ExitStack

    # Honest ACT per-instruction overhead for the tile scheduler's cost
    # model (measured ~352 cycles vs the default 172/222): the static
    # per-engine order then interleaves projection matmuls into the
    # ACT-gated attention stretches instead of stalling PE on exp.
    try:
        from concourse.hw_specs import TRN2Spec
        from concourse.bass import MemorySpace
        TRN2Spec.ACCESS_CYCLES[(MemorySpace.PSUM, mybir.EngineType.Activation)] = 352
        TRN2Spec.ACCESS_CYCLES[(MemorySpace.SBUF, mybir.EngineType.Activation)] = 352
    except Exception:
        pass

    nc = bacc.Bacc("TRN2", target_bir_lowering=False, debug=False,
                   num_devices=NCORES)
    dram = {
        "x8": nc.dram_tensor("x8", [NCH8, 128, 2, S], FP8,
                             kind="ExternalInput").ap(),
        "xb": nc.dram_tensor("xb", [8, 128, 512], BF16,
                             kind="ExternalInput").ap(),
        "wq8": nc.dram_tensor("wq8", [NCH8, 128, 2, FL], FP8,
                              kind="ExternalInput").ap(),
        "wk8": nc.dram_tensor("wk8", [NCH8, 128, 2, FL], FP8,
                              kind="ExternalInput").ap(),
        "wv8": nc.dram_tensor("wv8", [NCH8, 128, 2, FL], FP8,
                              kind="ExternalInput").ap(),
        "wvb": nc.dram_tensor("wvb", [8, 128, FL], BF16,
                              kind="ExternalInput").ap(),
        "wo8": nc.dram_tensor("wo8", [2, 128, 2, D], FP8,
                              kind="ExternalInput").ap(),
        "wob": nc.dram_tensor("wob", [NP, 128, D], BF16,
                              kind="ExternalInput").ap(),
        "sinq": nc.dram_tensor("sinq", [128, S], BF16,
                               kind="ExternalInput").ap(),
        "out": nc.dram_tensor("out", [S, D], F32, kind="ExternalOutput").ap(),
    }

    import os
    trace_sim = bool(os.environ.get("KTRACE"))
    with tile.TileContext(nc, trace_sim=trace_sim) as tc, ExitStack() as ctx:
        P = (
            ctx.enter_context(tc.tile_pool(name="x", bufs=2 * NCH8)),
            ctx.enter_context(tc.tile_pool(name="xb", bufs=8)),
            ctx.enter_context(tc.tile_pool(name="w", bufs=3 * NCH8 + 4)),
            ctx.enter_context(tc.tile_pool(name="wb", bufs=8)),
            ctx.enter_context(tc.tile_pool(name="wo", bufs=2)),
            ctx.enter_context(tc.tile_pool(name="wob", bufs=NP)),
            ctx.enter_context(tc.tile_pool(name="qk", bufs=2 * NP)),
            ctx.enter_context(tc.tile_pool(name="v", bufs=8)),
            ctx.enter_context(tc.tile_pool(name="vb", bufs=4)),
            ctx.enter_context(tc.tile_pool(name="sin", bufs=2)),
            ctx.enter_context(tc.tile_pool(name="pt", bufs=8)),
            ctx.enter_context(tc.tile_pool(name="ptb", bufs=8)),
            ctx.enter_context(tc.tile_pool(name="yt", bufs=2)),
            ctx.enter_context(tc.tile_pool(name="ytb", bufs=NP)),
            ctx.enter_context(tc.tile_pool(name="r", bufs=4)),
            ctx.enter_context(tc.tile_pool(name="os", bufs=4)),
            ctx.enter_context(tc.tile_pool(name="psp", bufs=2, space="PSUM")),
            ctx.enter_context(tc.tile_pool(name="pss", bufs=2, space="PSUM")),
            ctx.enter_context(tc.tile_pool(name="pso", bufs=2, space="PSUM")),
        )
        for rep in range(reps):
            _emit_body(nc, P, dram, rep)

    nc.finalize()
    _CACHED[key] = nc
    return nc


def _host_prep(x, wq, wk, wv, wo):
    """Fold RoPE rotation + fp8 scale into the weights; slice per core."""
    rope_dim = AOD // 2
    j = np.arange(rope_dim, dtype=np.float32)
    thetas = (1.0 / (10000.0 ** (2.0 * j / rope_dim))).astype(np.float32)
    pos = np.arange(S, dtype=np.float32)
    angles = pos[:, None] * thetas[None, :]          # [S, 32]
    sinv = np.sin(angles).astype(np.float32)         # [S, 32]
    # sin pattern tile [128, S]: row r multiplies feature (64*pair + r%64);
    # /16 descales the x128 fp8 weight scaling down to qt = 8*q
    sin2 = (np.tile(sinv.T, (4, 1)) / 16.0).astype(np.float32)

    def fold(w):
        wr = w.reshape(D, H, 2, rope_dim)
        return np.concatenate(
            [wr[:, :, 0] - wr[:, :, 1], wr[:, :, 0] + wr[:, :, 1]],
            axis=2).reshape(D, D)

    wqf = fold(wq) * 128.0
    wkf = fold(wk) * 128.0

    bf = ml_dtypes.bfloat16
    f8 = ml_dtypes.float8_e4m3

    def dr(w):  # [D, M] -> [NCH8, 128, 2, M] DoubleRow layout, fp8
        m = w.shape[1]
        return np.ascontiguousarray(
            w.reshape(NCH8, 2, 128, m).transpose(0, 2, 1, 3)).astype(f8)

    in_maps = []
    for core in range(NCORES):
        b, g = divmod(core, 2)
        sl = slice(g * FL, (g + 1) * FL)
        xT = np.ascontiguousarray(x[b].T)            # [D, S] f32
        wo_l = wo[sl, :] * 8.0                       # [FL, D]
        in_maps.append({
            "x8": dr(xT),
            "xb": np.ascontiguousarray(
                xT[:, 0:512].reshape(8, 128, 512)).astype(bf),
            "wq8": dr(wqf[:, sl]),
            "wk8": dr(wkf[:, sl]),
            "wv8": dr(wv[:, sl] * 128.0),
            "wvb": np.ascontiguousarray(
                (wv[:, sl] * 128.0).reshape(8, 128, FL)).astype(bf),
            "wo8": np.ascontiguousarray(
                wo_l.reshape(2, 2, 128, D).transpose(0, 2, 1, 3)).astype(f8),
            "wob": np.ascontiguousarray(
                wo_l.reshape(NP, 128, D)).astype(bf),
            "sinq": sin2.astype(bf),
        })
    return in_maps


def kernel(x, wq, wk, wv, wo, bo):
    nc = build_nc()
    in_maps = _host_prep(np.asarray(x, np.float32), np.asarray(wq, np.float32),
                         np.asarray(wk, np.float32), np.asarray(wv, np.float32),
                         np.asarray(wo, np.float32))
    res = run_bass_kernel_spmd(nc, in_maps, list(range(NCORES)))
    out = np.empty((B, S, D), np.float32)
    bo32 = np.asarray(bo, np.float32)
    for b in range(B):
        out[b] = res.results[2 * b]["out"] + res.results[2 * b + 1]["out"] + bo32
    return out


# revision 47
# speedup vs baseline: 1.3933x; 1.3933x over previous
"""Causal multi-head attention with (buggy-faithful) RoPE on 8 trn2 cores.

Problem: B=4, S=2048, D=1024, H=16 heads of dim 64, fp32.
Sharding: core c handles batch b=c//2 and head-group g=c%2 (8 heads).
Each core computes partial_out = attn(x_b, heads g) @ wo[rows g]; the host
sums the two partials per batch and adds the bias.

v2: fp8 (e4m3) DoubleRow matmuls for the PE-heavy stages. Double FP8 packs
two contraction rows per partition (K_eff=256) for 2x tensor-engine
throughput. Precision plan (validated host-side, rel err ~2e-3):
- Q/K projections: fp8 DoubleRow (q/k noise ~4% is washed out by softmax).
- V projection: bf16 for kv<512, fp8 DoubleRow for kv>=512.
- P@V + the attention probs pt: fp8 for q>=512 (t>=1), bf16 for t=0. Early
  rows have peaked attention (y ~= v_k), so V/pt quantization error there
  lands directly in the output max; rows>=512 average over >=512 positions.
- Output projection: fp8 DoubleRow for q>=512, bf16 for t=0.
- Scores stay bf16 (K=64 per head; two heads pack PE row-groups 0-1/2-3
  concurrently via tile_position, so bf16 scores already run at full rate).

Scale folding keeps every fp8 tensor in e4m3's happy range (max 240):
wq/wk *128 (host) with /16 folded into the bf16 sin table -> qt,kt = 8*q,8*k
(exp scale 0.125/64); wv *128 with *1/8 in the drain -> v16 = 16*v; ones
column 1.0 -> yt = 16*y; wo *8 -> out psum = 128*out, *1/128 in the drain.

On-device layout (per core):
- x8 [4][128, 2, S] fp8: feature d = 256c+128i+p (DoubleRow pair on i).
- QT/KT [512, 2048] bf16 feature-major; V as [128, 2, 8, 65] fp8 per 256-kv
  chunk (65th col = ones -> softmax denominator accumulates in the P@V
  matmul) + bf16 [128, 8, 65] for kv<512.
- S^T [kv, q] per head; exp needs no max subtraction (|scores/8| < ~3).
- pt8 [128, 2, 2, 512] fp8 per 256-kv chunk: (kv-half i, parity e, q).
- P@V full 256-chunks: one DoubleRow matmul per head; diagonal 128-chunks:
  plain fp8 matmuls over the trapezoid, mask boundary via affine_select.
"""

import numpy as np
import ml_dtypes

import concourse.bacc as bacc
import concourse.mybir as mybir
import concourse.tile as tile
from concourse.bass_utils import run_bass_kernel_spmd

B, S, D = 4, 2048, 1024
H = 16
AOD = 64
HL = 8            # heads per core
FL = HL * AOD     # 512 local features
NCORES = 8
NCH8 = 4          # fp8 DoubleRow contraction chunks (256 features each)
NSQ = 4           # sq tiles of 512
NP = FL // 128    # 4 feature chunks (head pairs)
NT16 = S // 128   # 16 seq chunks of 128

F32 = mybir.dt.float32
BF16 = mybir.dt.bfloat16
FP8 = mybir.dt.float8e4
EXP = mybir.ActivationFunctionType.Exp
DR = mybir.MatmulPerfMode.DoubleRow
EXP_SCALE = 0.125 / 64.0   # qt,kt carry x8 each

_CACHED = {}


def _alu():
    from concourse.alu_op_type import AluOpType
    return AluOpType


def _emit_vproj(nc, P, sb, q, rep):
    """V projection for seq tile q (128 positions): writes v8 (fp8, 16*v)
    and, for q<4, also vb (bf16, 16*v) for the t=0 path."""
    (p_x, p_xb, p_w, p_wb, p_wo, p_wob, p_qk, p_v, p_vb, p_sin, p_pt,
     p_ptb, p_yt, p_ytb, p_r, p_os, ps_proj, ps_s, ps_o) = P
    ps = ps_proj.tile([128, FL], F32, tag="psp", name=f"r{rep}vps{q}")
    if q < 4:
        for c in range(8):
            nc.tensor.matmul(
                ps[:], sb["xb"][c][:, 128 * q:128 * (q + 1)], sb["wvb"][c][:],
                start=(c == 0), stop=(c == 7))
    else:
        for c in range(NCH8):
            nc.tensor.matmul(
                ps[:], sb["x8"][c][:, :, 128 * q:128 * (q + 1)],
                sb["wv8"][c][:], perf_mode=DR,
                start=(c == 0), stop=(c == NCH8 - 1))
    nc.vector.tensor_scalar_mul(
        sb["v8"][q // 2][:, q % 2, :, 0:64],
        ps[:].rearrange("p (h d) -> p h d", h=HL), 0.125)
    if q < 4:
        nc.vector.tensor_scalar_mul(
            sb["vb"][q][:, :, 0:64],
            ps[:].rearrange("p (h d) -> p h d", h=HL), 0.125)


def _emit_body(nc, P, dram, rep):
    """One full forward pass for this core's shard."""
    mult = _alu().mult
    is_ge = _alu().is_ge
    (p_x, p_xb, p_w, p_wb, p_wo, p_wob, p_qk, p_v, p_vb, p_sin, p_pt,
     p_ptb, p_yt, p_ytb, p_r, p_os, ps_proj, ps_s, ps_o) = P
    R = f"r{rep}"

    # ---- resident loads, split across the qSP and qPool DMA queues so the
    # first-needed tensors land in ~3us instead of ~17us serial. The Pool
    # queue only carries the early small loads: gpsimd DMA blocks the Pool
    # engine for the transfer, and Pool must stay free for the causal-mask
    # affine_selects in the exp->PV chain. ----
    sb = {}
    # warm the ACT exp table during the DMA window (saves ~1.4us at first
    # exp); the table persists, so only the first rep needs it
    if rep == 0:
        warm = p_r.tile([1, 16], F32, tag="warm", name=f"{R}warm")
        nc.gpsimd.memset(warm[:], 0.0)
        nc.scalar.activation(warm[:], warm[:], EXP, scale=1.0)
    sb["wq8"] = []
    sb["wk8"] = []
    for c in range(NCH8):
        t = p_w.tile([128, 2, FL], FP8, tag="w", name=f"{R}wq8{c}")
        (nc.sync if c < 2 else nc.gpsimd).dma_start(t[:], dram["wq8"][c])
        sb["wq8"].append(t)
    sb["x8"] = [p_x.tile([128, 2, S], FP8, tag="x", name=f"{R}x{c}")
                for c in range(NCH8)]
    for c in range(NCH8):   # t=0 column slice first: compute starts early
        nc.sync.dma_start(
            sb["x8"][c][:, :, 0:512], dram["x8"][c, :, :, 0:512])
    for c in range(NCH8):
        t = p_w.tile([128, 2, FL], FP8, tag="w", name=f"{R}wk8{c}")
        nc.gpsimd.dma_start(t[:], dram["wk8"][c])
        sb["wk8"].append(t)
    sin_sb = p_sin.tile([128, S], BF16, tag="sin", name=f"{R}sin_sb")
    nc.gpsimd.dma_start(sin_sb[:], dram["sinq"])
    for c in range(2):
        nc.sync.dma_start(
            sb["x8"][c][:, :, 512:1024], dram["x8"][c, :, :, 512:1024])
    sb["xb"] = []
    for c in range(8):
        t = p_xb.tile([128, 512], BF16, tag="xb", name=f"{R}xb{c}")
        (nc.sync if c < 4 else nc.gpsimd).dma_start(t[:], dram["xb"][c])
        sb["xb"].append(t)
    sb["wvb"] = []
    for c in range(8):
        t = p_wb.tile([128, FL], BF16, tag="wb", name=f"{R}wvb{c}")
        (nc.sync if c < 4 else nc.gpsimd).dma_start(t[:], dram["wvb"][c])
        sb["wvb"].append(t)
    for c in range(2, NCH8):
        nc.gpsimd.dma_start(
            sb["x8"][c][:, :, 512:1024], dram["x8"][c, :, :, 512:1024])
    for c in range(NCH8):
        (nc.sync if c < 2 else nc.gpsimd).dma_start(
            sb["x8"][c][:, :, 1024:1536], dram["x8"][c, :, :, 1024:1536])
    sb["wv8"] = []
    for c in range(NCH8):
        t = p_w.tile([128, 2, FL], FP8, tag="w", name=f"{R}wv8{c}")
        nc.sync.dma_start(t[:], dram["wv8"][c])
        sb["wv8"].append(t)
    for c in range(NCH8):
        nc.sync.dma_start(
            sb["x8"][c][:, :, 1536:2048], dram["x8"][c, :, :, 1536:2048])
    sb["wo8"] = []
    for c in range(2):
        t = p_wo.tile([128, 2, D], FP8, tag="wo", name=f"{R}wo8{c}")
        nc.sync.dma_start(t[:], dram["wo8"][c])
        sb["wo8"].append(t)
    sb["wob"] = []
    for c in range(NP):
        t = p_wob.tile([128, D], BF16, tag="wob", name=f"{R}wob{c}")
        nc.sync.dma_start(t[:], dram["wob"][c])
        sb["wob"].append(t)

    # V tiles: fp8 per 256-kv chunk (+ bf16 for kv<512); ones col up front
    sb["v8"] = []
    for c in range(8):
        # 66-wide per head: DoubleRow LDWEIGHTS needs the pair-dim byte
        # step (8*66=528) to be 16-aligned; col 64 = ones, col 65 = pad
        vt = p_v.tile([128, 2, HL, 66], FP8, tag="v", name=f"{R}v8_{c}")
        nc.gpsimd.memset(vt[:, :, :, 64:65], 1.0)
        sb["v8"].append(vt)
    sb["vb"] = []
    for c in range(4):
        vt = p_vb.tile([128, HL, 65], BF16, tag="vb", name=f"{R}vb{c}")
        nc.gpsimd.memset(vt[:, :, 64:65], 1.0)
        sb["vb"].append(vt)

    # yt: bf16 [128, 512] per pair for t=0; fp8 DoubleRow layout for t>=1
    sb["ytb"] = [p_ytb.tile([128, 512], BF16, tag="ytb", name=f"{R}ytb{i}")
                 for i in range(NP)]
    sb["yt8"] = [p_yt.tile([128, 2, 1536], FP8, tag="yt", name=f"{R}yt8{i}")
                 for i in range(2)]
    qt_sb = [p_qk.tile([128, S], BF16, tag="qk", name=f"{R}qt{i}")
             for i in range(NP)]
    kt_sb = [p_qk.tile([128, S], BF16, tag="qk", name=f"{R}kt{i}")
             for i in range(NP)]

    def emit_qkproj(p, t):
        # Q and K projection slice (fp8 DoubleRow), rope fused into the drain
        for w_sb, dst, nm in ((sb["wq8"], qt_sb[p], "q"),
                              (sb["wk8"], kt_sb[p], "k")):
            ps = ps_proj.tile([128, 512], F32, tag="psp",
                              name=f"{R}{nm}ps{p}{t}")
            for c in range(NCH8):
                nc.tensor.matmul(
                    ps[:],
                    w_sb[c][:, :, 128 * p:128 * (p + 1)],
                    sb["x8"][c][:, :, 512 * t:512 * (t + 1)],
                    perf_mode=DR,
                    start=(c == 0), stop=(c == NCH8 - 1))
            if p == 0 and t == 0 and nm == "k":
                # cold start: the first score chunk only reads kt[:, 0:128];
                # a split drain lets it go ~0.5us earlier
                nc.vector.tensor_tensor(
                    out=dst[:, 0:128], in0=ps[:, 0:128],
                    in1=sin_sb[:, 0:128], op=mult)
                nc.vector.tensor_tensor(
                    out=dst[:, 128:512], in0=ps[:, 128:512],
                    in1=sin_sb[:, 128:512], op=mult)
            else:
                nc.vector.tensor_tensor(
                    out=dst[:, 512 * t:512 * (t + 1)],
                    in0=ps[:], in1=sin_sb[:, 512 * t:512 * (t + 1)],
                    op=mult)

    emit_qkproj(0, 0)
    # attention for heads 2p (e=0) and 2p+1 (e=1). Scores for both parities
    # share one [128, 2, 512] S^T psum tile; their K=64 matmuls pack PE
    # row-groups 0-1/2-3 concurrently, and one exp covers both parities.
    # P@V runs one chunk behind the S matmuls so PE never waits on ACT.
    # Block order: pairs {0,1} interleaved over t first — two pairs' early
    # exp work covers the DMA window for the later x slices — then p-outer
    # for {2,3} (long blocks keep the psum-pool recycling off the exp path).
    SEQ = ([(0, 0), (0, 1), (1, 0), (1, 1), (0, 2), (1, 2), (0, 3), (1, 3)]
           + [(p, t) for p in (2, 3) for t in range(NSQ)])

    def emit_pv0(pp, o_ps_l, pts_l):
        for cp, offp, ptp in pts_l:
            for e in range(2):
                nc.tensor.matmul(
                    o_ps_l[e][:, offp:512], sb["vb"][cp][:, 2 * pp + e, :],
                    ptp[:, e, offp:512],
                    start=(cp == 0), stop=(cp == 3))

    def emit_norm(pp, tt, o_ps_l):
        # recip of the ones-row, broadcast, scale the 64 rows. Last tile
        # runs in 256-wide halves so the tail O-proj starts as soon as the
        # first half of every pair's yt lands.
        halves = ((0, 256), (256, 512)) if tt == NSQ - 1 else ((0, 512),)
        for h0, h1 in halves:
            for e in range(2):
                recip = p_r.tile([1, h1 - h0], F32, tag="rc",
                                 name=f"{R}rc{pp}_{tt}{e}_{h0}")
                nc.vector.reciprocal(recip[:], o_ps_l[e][64:65, h0:h1])
                rb = p_r.tile([64, h1 - h0], F32, tag="rb",
                              name=f"{R}rb{pp}_{tt}{e}_{h0}")
                nc.gpsimd.partition_broadcast(rb[:], recip[:], channels=64)
                if tt == 0:
                    dst = sb["ytb"][pp][64 * e:64 * (e + 1), h0:h1]
                else:
                    dst = sb["yt8"][pp // 2][64 * e:64 * (e + 1), pp % 2,
                                             512 * tt - 512 + h0:
                                             512 * tt - 512 + h1]
                nc.vector.tensor_tensor(
                    out=dst, in0=o_ps_l[e][0:64, h0:h1], in1=rb[:], op=mult)

    def emit_oproj_unit(q, o, tt):
        # tail units alternate through the pso pool (idle after the last
        # norm) so psp frees early for the next rep's first projections
        pool = ps_o if (tt == NSQ - 1 and o == 1) else ps_proj
        ps = pool.tile([128, 512], F32, tag="psp" if pool is ps_proj else "pso",
                       name=f"{R}ops{q}{o}")
        if tt == 0:
            for pp in range(NP):
                nc.tensor.matmul(
                    ps[:],
                    sb["ytb"][pp][:, 128 * q:128 * (q + 1)],
                    sb["wob"][pp][:, 512 * o:512 * (o + 1)],
                    start=(pp == 0), stop=(pp == NP - 1))
        else:
            for c2 in range(2):
                nc.tensor.matmul(
                    ps[:],
                    sb["yt8"][c2][:, :, 128 * q - 512:128 * (q + 1) - 512],
                    sb["wo8"][c2][:, :, 512 * o:512 * (o + 1)],
                    perf_mode=DR, start=(c2 == 0), stop=(c2 == 1))
        os_t = p_os.tile([128, 512], F32, tag="os", name=f"{R}os{q}{o}")
        if tt == NSQ - 1:
            # tail: ACT idles after the last exp and DVE after the last
            # yt; split the drains. Pool's DMA queue has no selects left.
            if o == 0:
                nc.scalar.mul(os_t[:], ps[:], 1.0 / 128.0)
            else:
                nc.vector.tensor_scalar_mul(os_t[:], ps[:], 1.0 / 128.0)
            eng = nc.sync if o == 0 else nc.gpsimd
        else:
            nc.vector.tensor_scalar_mul(os_t[:], ps[:], 1.0 / 128.0)
            eng = nc.sync
        eng.dma_start(
            dram["out"][128 * q:128 * (q + 1), 512 * o:512 * (o + 1)],
            os_t[:])

    def emit_pv8_flush(pp, tt, o_ps_l, ptt, pvq):
        n = len(pvq)
        for idx, (kind, cc2, _) in enumerate(pvq):
            first, last = idx == 0, idx == n - 1
            if kind == "full":
                ptp = ptt[("f", cc2)]
                for e in range(2):
                    nc.tensor.matmul(
                        o_ps_l[e][:, :], sb["v8"][cc2][:, :, 2 * pp + e, 0:65],
                        ptp[:, :, e, :], perf_mode=DR, start=first, stop=last)
            else:
                c = cc2
                off = 128 * (c - 4 * tt)
                ptp = ptt[("f", c // 2)]
                for e in range(2):
                    nc.tensor.matmul(
                        o_ps_l[e][:, off:512],
                        sb["v8"][c // 2][:, c % 2, 2 * pp + e, 0:65],
                        ptp[:, c % 2, e, off:512], start=first, stop=last)

    hold00 = []
    hold01 = []
    oproj_pending = []
    for bi, (p, t) in enumerate(SEQ):
        if True:
            nxt = SEQ[bi + 1] if bi + 1 < len(SEQ) else None
            o_ps = [ps_o.tile([65, 512], F32, tag="pso",
                              name=f"{R}o{p}_{t}_{i}") for i in range(2)]

            if t == 0:
                # bf16 path (4 diagonal 128-chunks). vproj/PV emitted after
                # the score/exp stream: their xb/wvb DMAs land late and must
                # not clog the PE wait queue ahead of the scores. Pair 0's
                # PV+norm are deferred into pair 1's block for the same
                # reason.
                pts = []
                for c in range(4):
                    off = 128 * c
                    w = 512 - off
                    s_ps = ps_s.tile([128, 2, 512], F32, tag="s",
                                     name=f"{R}s{p}_0_{c}")
                    for e in range(2):
                        nc.tensor.matmul(
                            s_ps[:, e, 0:w],
                            kt_sb[p][64 * e:64 * (e + 1),
                                     128 * c:128 * (c + 1)],
                            qt_sb[p][64 * e:64 * (e + 1), off:512],
                            start=True, stop=True)
                    if c == 1 and nxt is not None:
                        emit_qkproj(*nxt)
                    pt = p_ptb.tile([128, 2, 512], BF16, tag="ptb",
                                    name=f"{R}pt{p}_0_{c}")
                    if off:
                        nc.scalar.activation(
                            pt[:, :, off:512], s_ps[:, :, 0:w],
                            EXP, scale=EXP_SCALE)
                    else:
                        nc.scalar.activation(
                            pt[:].rearrange("p a b -> p (a b)"),
                            s_ps[:].rearrange("p a b -> p (a b)"),
                            EXP, scale=EXP_SCALE)
                    nc.gpsimd.affine_select(
                        out=pt[:, :, off:off + 128],
                        in_=pt[:, :, off:off + 128],
                        compare_op=is_ge, fill=0.0, base=0,
                        pattern=[[0, 2], [1, 128]], channel_multiplier=-1)
                    pts.append((c, off, pt))
                if p == 0:
                    hold00.append((o_ps, pts))
                    continue
                if p == 1 and hold00:
                    # the full deferred train: vb/v8 projections (late DMA)
                    # plus pair 0's t0 and t1 PV+norm, all emitted behind
                    # this block's exp stream so no block's scores ever sit
                    # behind it in the PE queue
                    for q in range(8):
                        _emit_vproj(nc, P, sb, q, rep)
                    o_ps00, pts00 = hold00.pop()
                    emit_pv0(0, o_ps00, pts00)
                    emit_norm(0, 0, o_ps00)
                    o_ps01, ptt01, pvq01 = hold01.pop()
                    emit_pv8_flush(0, 1, o_ps01, ptt01, pvq01)
                    emit_norm(0, 1, o_ps01)
                emit_pv0(p, o_ps, pts)
            else:
                # fp8 path: full 256-chunks DoubleRow, diagonal 128-chunks
                # plain fp8. pt8 per 256-chunk: [128, 2(kv-half), 2(par), 512].
                pt_tiles = {}
                pv_queue = []   # (kind, c2_or_c, gate_cc)
                for c2 in range(2 * t):
                    pv_queue.append(("full", c2, 2 * c2 + 1))
                for c in range(4 * t, 4 * (t + 1)):
                    pv_queue.append(("diag", c, c))
                n_pv = len(pv_queue)
                emitted = [0]

                def emit_pv8(stop_at):
                    while emitted[0] < stop_at:
                        kind, cc2, _ = pv_queue[emitted[0]]
                        first = emitted[0] == 0
                        last = emitted[0] == n_pv - 1
                        if kind == "full":
                            ptp = pt_tiles[("f", cc2)]
                            for e in range(2):
                                nc.tensor.matmul(
                                    o_ps[e][:, :],
                                    sb["v8"][cc2][:, :, 2 * p + e, 0:65],
                                    ptp[:, :, e, :], perf_mode=DR,
                                    start=first, stop=last)
                        else:
                            c = cc2
                            off = 128 * (c - 4 * t)
                            ptp = pt_tiles[("f", c // 2)]
                            for e in range(2):
                                nc.tensor.matmul(
                                    o_ps[e][:, off:512],
                                    sb["v8"][c // 2][:, c % 2, 2 * p + e, 0:65],
                                    ptp[:, c % 2, e, off:512],
                                    start=first, stop=last)
                        emitted[0] += 1

                for cc in range(4 * (t + 1)):
                    c2, i = cc // 2, cc % 2
                    dc = cc - 4 * t
                    off = 128 * dc if dc > 0 else 0
                    w = 512 - off
                    s_ps = ps_s.tile([128, 2, 512], F32, tag="s",
                                     name=f"{R}s{p}_{t}_{cc}")
                    for e in range(2):
                        nc.tensor.matmul(
                            s_ps[:, e, 0:w],
                            kt_sb[p][64 * e:64 * (e + 1),
                                     128 * cc:128 * (cc + 1)],
                            qt_sb[p][64 * e:64 * (e + 1),
                                     512 * t + off:512 * (t + 1)],
                            start=True, stop=True)
                    if p == 0 and cc < 4 and bi != 1:
                        _emit_vproj(nc, P, sb, 4 * t + cc, rep)
                    if cc == 1 and nxt is not None:
                        emit_qkproj(*nxt)
                    if ("f", c2) not in pt_tiles:
                        pt_tiles[("f", c2)] = p_pt.tile(
                            [128, 2, 2, 512], FP8, tag="pt",
                            name=f"{R}pt{p}_{t}_{c2}")
                    pt = pt_tiles[("f", c2)]
                    if off:
                        nc.scalar.activation(
                            pt[:, i, :, off:512], s_ps[:, :, 0:w],
                            EXP, scale=EXP_SCALE)
                    else:
                        nc.scalar.activation(
                            pt[:, i, :, :].rearrange("p a b -> p (a b)"),
                            s_ps[:].rearrange("p a b -> p (a b)"),
                            EXP, scale=EXP_SCALE)
                    if dc >= 0:
                        # causal boundary within [off:off+128]
                        nc.gpsimd.affine_select(
                            out=pt[:, i, :, off:off + 128],
                            in_=pt[:, i, :, off:off + 128],
                            compare_op=is_ge, fill=0.0, base=0,
                            pattern=[[0, 2], [1, 128]], channel_multiplier=-1)
                    # emit PV units whose pt is complete, one chunk behind
                    if bi != 1:
                        ready = sum(1 for u in pv_queue if u[2] <= cc - 1)
                        emit_pv8(ready)
                    if oproj_pending:
                        emit_oproj_unit(*oproj_pending.pop(0))
                if bi == 1:
                    # stash: this block's PV depends on the vproj train
                    # (late DMA); both are deferred into block (1,0)
                    hold01.append((o_ps, pt_tiles, pv_queue))
                else:
                    emit_pv8(n_pv)

            if bi == 1:
                continue
            emit_norm(p, t, o_ps)

            if p == NP - 1:
                # output projection for the seq tiles this t completes.
                # t<3 units are queued and interleaved into the NEXT block's
                # chunk loop so their matmuls/drains don't sit ahead of that
                # block's scores in the PE/DVE streams; t=3 is the tail.
                units = [(q, o, t) for q in range(4 * t, 4 * (t + 1))
                         for o in range(2)]
                if t < NSQ - 1:
                    oproj_pending.extend(units)
                else:
                    while oproj_pending:
                        emit_oproj_unit(*oproj_pending.pop(0))
                    for u in units:
                        emit_oproj_unit(*u)


def build_nc(reps=1):
    key = ("nc", reps)
    if key in _CACHED:
        return _CACHED[key]
    from contextlib import ExitStack

    # Honest ACT per-instruction overhead for the tile scheduler's cost
    # model (measured ~352 cycles vs the default 172/222): the static
    # per-engine order then interleaves projection matmuls into the
    # ACT-gated attention stretches instead of stalling PE on exp.
    try:
        from concourse.hw_specs import TRN2Spec
        from concourse.bass import MemorySpace
        TRN2Spec.ACCESS_CYCLES[(MemorySpace.PSUM, mybir.EngineType.Activation)] = 352
        TRN2Spec.ACCESS_CYCLES[(MemorySpace.SBUF, mybir.EngineType.Activation)] = 352
    except Exception:
        pass

    nc = bacc.Bacc("TRN2", target_bir_lowering=False, debug=False,
                   num_devices=NCORES)
    dram = {
        "x8": nc.dram_tensor("x8", [NCH8, 128, 2, S], FP8,
                             kind="ExternalInput").ap(),
        "xb": nc.dram_tensor("xb", [8, 128, 512], BF16,
                             kind="ExternalInput").ap(),
        "wq8": nc.dram_tensor("wq8", [NCH8, 128, 2, FL], FP8,
                              kind="ExternalInput").ap(),
        "wk8": nc.dram_tensor("wk8", [NCH8, 128, 2, FL], FP8,
                              kind="ExternalInput").ap(),
        "wv8": nc.dram_tensor("wv8", [NCH8, 128, 2, FL], FP8,
                              kind="ExternalInput").ap(),
        "wvb": nc.dram_tensor("wvb", [8, 128, FL], BF16,
                              kind="ExternalInput").ap(),
        "wo8": nc.dram_tensor("wo8", [2, 128, 2, D], FP8,
                              kind="ExternalInput").ap(),
        "wob": nc.dram_tensor("wob", [NP, 128, D], BF16,
                              kind="ExternalInput").ap(),
        "sinq": nc.dram_tensor("sinq", [128, S], BF16,
                               kind="ExternalInput").ap(),
        "out": nc.dram_tensor("out", [S, D], F32, kind="ExternalOutput").ap(),
    }

    import os
    trace_sim = bool(os.environ.get("KTRACE"))
    with tile.TileContext(nc, trace_sim=trace_sim) as tc, ExitStack() as ctx:
        P = (
            ctx.enter_context(tc.tile_pool(name="x", bufs=2 * NCH8)),
            ctx.enter_context(tc.tile_pool(name="xb", bufs=8)),
            ctx.enter_context(tc.tile_pool(name="w", bufs=3 * NCH8 + 4)),
            ctx.enter_context(tc.tile_pool(name="wb", bufs=8)),
            ctx.enter_context(tc.tile_pool(name="wo", bufs=2)),
            ctx.enter_context(tc.tile_pool(name="wob", bufs=NP)),
            ctx.enter_context(tc.tile_pool(name="qk", bufs=2 * NP)),
            ctx.enter_context(tc.tile_pool(name="v", bufs=8)),
            ctx.enter_context(tc.tile_pool(name="vb", bufs=4)),
            ctx.enter_context(tc.tile_pool(name="sin", bufs=2)),
            ctx.enter_context(tc.tile_pool(name="pt", bufs=8)),
            ctx.enter_context(tc.tile_pool(name="ptb", bufs=8)),
            ctx.enter_context(tc.tile_pool(name="yt", bufs=2)),
            ctx.enter_context(tc.tile_pool(name="ytb", bufs=NP)),
            ctx.enter_context(tc.tile_pool(name="r", bufs=4)),
            ctx.enter_context(tc.tile_pool(name="os", bufs=4)),
            ctx.enter_context(tc.tile_pool(name="psp", bufs=2, space="PSUM")),
            ctx.enter_context(tc.tile_pool(name="pss", bufs=2, space="PSUM")),
            ctx.enter_context(tc.tile_pool(name="pso", bufs=2, space="PSUM")),
        )
        for rep in range(reps):
            _emit_body(nc, P, dram, rep)

    nc.finalize()
    _CACHED[key] = nc
    return nc


def _host_prep(x, wq, wk, wv, wo):
    """Fold RoPE rotation + fp8 scale into the weights; slice per core."""
    rope_dim = AOD // 2
    j = np.arange(rope_dim, dtype=np.float32)
    thetas = (1.0 / (10000.0 ** (2.0 * j / rope_dim))).astype(np.float32)
    pos = np.arange(S, dtype=np.float32)
    angles = pos[:, None] * thetas[None, :]          # [S, 32]
    sinv = np.sin(angles).astype(np.float32)         # [S, 32]
    # sin pattern tile [128, S]: row r multiplies feature (64*pair + r%64);
    # /16 descales the x128 fp8 weight scaling down to qt = 8*q
    sin2 = (np.tile(sinv.T, (4, 1)) / 16.0).astype(np.float32)

    def fold(w):
        wr = w.reshape(D, H, 2, rope_dim)
        return np.concatenate(
            [wr[:, :, 0] - wr[:, :, 1], wr[:, :, 0] + wr[:, :, 1]],
            axis=2).reshape(D, D)

    wqf = fold(wq) * 128.0
    wkf = fold(wk) * 128.0

    bf = ml_dtypes.bfloat16
    f8 = ml_dtypes.float8_e4m3

    def dr(w):  # [D, M] -> [NCH8, 128, 2, M] DoubleRow layout, fp8
        m = w.shape[1]
        return np.ascontiguousarray(
            w.reshape(NCH8, 2, 128, m).transpose(0, 2, 1, 3)).astype(f8)

    in_maps = []
    for core in range(NCORES):
        b, g = divmod(core, 2)
        sl = slice(g * FL, (g + 1) * FL)
        xT = np.ascontiguousarray(x[b].T)            # [D, S] f32
        wo_l = wo[sl, :] * 8.0                       # [FL, D]
        in_maps.append({
            "x8": dr(xT),
            "xb": np.ascontiguousarray(
                xT[:, 0:512].reshape(8, 128, 512)).astype(bf),
            "wq8": dr(wqf[:, sl]),
            "wk8": dr(wkf[:, sl]),
            "wv8": dr(wv[:, sl] * 128.0),
            "wvb": np.ascontiguousarray(
                (wv[:, sl] * 128.0).reshape(8, 128, FL)).astype(bf),
            "wo8": np.ascontiguousarray(
                wo_l.reshape(2, 2, 128, D).transpose(0, 2, 1, 3)).astype(f8),
            "wob": np.ascontiguousarray(
                wo_l.reshape(NP, 128, D)).astype(bf),
            "sinq": sin2.astype(bf),
        })
    return in_maps


def kernel(x, wq, wk, wv, wo, bo):
    nc = build_nc()
    in_maps = _host_prep(np.asarray(x, np.float32), np.asarray(wq, np.float32),
                         np.asarray(wk, np.float32), np.asarray(wv, np.float32),
                         np.asarray(wo, np.float32))
    res = run_bass_kernel_spmd(nc, in_maps, list(range(NCORES)))
    out = np.empty((B, S, D), np.float32)
    bo32 = np.asarray(bo, np.float32)
    for b in range(B):
        out[b] = res.results[2 * b]["out"] + res.results[2 * b + 1]["out"] + bo32
    return out


# revision 48
# speedup vs baseline: 1.7342x; 1.2447x over previous
"""Causal multi-head attention with (buggy-faithful) RoPE on 8 trn2 cores.

Problem: B=4, S=2048, D=1024, H=16 heads of dim 64, fp32.
Sharding: core c handles batch b=c//2 and head-group g=c%2 (8 heads).
Each core computes partial_out = attn(x_b, heads g) @ wo[rows g]; the host
sums the two partials per batch and adds the bias.

v2: fp8 (e4m3) DoubleRow matmuls for the PE-heavy stages. Double FP8 packs
two contraction rows per partition (K_eff=256) for 2x tensor-engine
throughput. Precision plan (validated host-side, rel err ~2e-3):
- Q/K projections: fp8 DoubleRow (q/k noise ~4% is washed out by softmax).
- V projection: bf16 for kv<512, fp8 DoubleRow for kv>=512.
- P@V + the attention probs pt: fp8 for q>=512 (t>=1), bf16 for t=0. Early
  rows have peaked attention (y ~= v_k), so V/pt quantization error there
  lands directly in the output max; rows>=512 average over >=512 positions.
- Output projection: fp8 DoubleRow for q>=512, bf16 for t=0.
- Scores stay bf16 (K=64 per head; two heads pack PE row-groups 0-1/2-3
  concurrently via tile_position, so bf16 scores already run at full rate).

Scale folding keeps every fp8 tensor in e4m3's happy range (max 240):
wq/wk *128 (host) with /16 folded into the bf16 sin table -> qt,kt = 8*q,8*k
(exp scale 0.125/64); wv *128 with *1/8 in the drain -> v16 = 16*v; ones
column 1.0 -> yt = 16*y; wo *8 -> out psum = 128*out, *1/128 in the drain.

On-device layout (per core):
- x8 [4][128, 2, S] fp8: feature d = 256c+128i+p (DoubleRow pair on i).
- QT/KT [512, 2048] bf16 feature-major; V as [128, 2, 8, 65] fp8 per 256-kv
  chunk (65th col = ones -> softmax denominator accumulates in the P@V
  matmul) + bf16 [128, 8, 65] for kv<512.
- S^T [kv, q] per head; exp needs no max subtraction (|scores/8| < ~3).
- pt8 [128, 2, 2, 512] fp8 per 256-kv chunk: (kv-half i, parity e, q).
- P@V full 256-chunks: one DoubleRow matmul per head; diagonal 128-chunks:
  plain fp8 matmuls over the trapezoid, mask boundary via affine_select.
"""

import numpy as np
import ml_dtypes

import concourse.bacc as bacc
import concourse.mybir as mybir
import concourse.tile as tile
from concourse.bass_utils import run_bass_kernel_spmd

B, S, D = 4, 2048, 1024
H = 16
AOD = 64
HL = 8            # heads per core
FL = HL * AOD     # 512 local features
NCORES = 8
NCH8 = 4          # fp8 DoubleRow contraction chunks (256 features each)
NSQ = 4           # sq tiles of 512
NP = FL // 128    # 4 feature chunks (head pairs)
NT16 = S // 128   # 16 seq chunks of 128

F32 = mybir.dt.float32
BF16 = mybir.dt.bfloat16
FP8 = mybir.dt.float8e4
EXP = mybir.ActivationFunctionType.Exp
DR = mybir.MatmulPerfMode.DoubleRow
EXP_SCALE = 0.125 / 64.0   # qt,kt carry x8 each

_CACHED = {}


def _alu():
    from concourse.alu_op_type import AluOpType
    return AluOpType


def _emit_vproj(nc, P, sb, q, rep):
    """V projection for seq tile q (128 positions): writes v8 (fp8, 16*v)
    and, for q<4, also vb (bf16, 16*v) for the t=0 path."""
    (p_x, p_xb, p_w, p_wb, p_wo, p_wob, p_qk, p_v, p_vb, p_sin, p_pt,
     p_ptb, p_yt, p_ytb, p_r, p_os, ps_proj, ps_s, ps_o) = P
    ps = ps_proj.tile([128, FL], F32, tag="psp", name=f"r{rep}vps{q}")
    if q < 4:
        for c in range(8):
            nc.tensor.matmul(
                ps[:], sb["xb"][c][:, 128 * q:128 * (q + 1)], sb["wvb"][c][:],
                start=(c == 0), stop=(c == 7))
    else:
        for c in range(NCH8):
            nc.tensor.matmul(
                ps[:], sb["x8"][c][:, :, 128 * q:128 * (q + 1)],
                sb["wv8"][c][:], perf_mode=DR,
                start=(c == 0), stop=(c == NCH8 - 1))
    nc.vector.tensor_scalar_mul(
        sb["v8"][q // 2][:, q % 2, :, 0:64],
        ps[:].rearrange("p (h d) -> p h d", h=HL), 0.125)
    if q < 4:
        nc.vector.tensor_scalar_mul(
            sb["vb"][q][:, :, 0:64],
            ps[:].rearrange("p (h d) -> p h d", h=HL), 0.125)


def _emit_body(nc, P, dram, rep):
    """One full forward pass for this core's shard."""
    mult = _alu().mult
    is_ge = _alu().is_ge
    (p_x, p_xb, p_w, p_wb, p_wo, p_wob, p_qk, p_v, p_vb, p_sin, p_pt,
     p_ptb, p_yt, p_ytb, p_r, p_os, ps_proj, ps_s, ps_o) = P
    R = f"r{rep}"

    # ---- resident loads, split across the qSP and qPool DMA queues so the
    # first-needed tensors land in ~3us instead of ~17us serial. The Pool
    # queue only carries the early small loads: gpsimd DMA blocks the Pool
    # engine for the transfer, and Pool must stay free for the causal-mask
    # affine_selects in the exp->PV chain. ----
    sb = {}
    # warm the ACT exp table during the DMA window (saves ~1.4us at first
    # exp); the table persists, so only the first rep needs it
    if rep == 0:
        warm = p_r.tile([1, 16], F32, tag="warm", name=f"{R}warm")
        nc.gpsimd.memset(warm[:], 0.0)
        nc.scalar.activation(warm[:], warm[:], EXP, scale=1.0)
    sb["wq8"] = []
    sb["wk8"] = []
    for c in range(NCH8):
        t = p_w.tile([128, 2, FL], FP8, tag="w", name=f"{R}wq8{c}")
        (nc.sync if c < 2 else nc.gpsimd).dma_start(t[:], dram["wq8"][c])
        sb["wq8"].append(t)
    sb["x8"] = [p_x.tile([128, 2, S], FP8, tag="x", name=f"{R}x{c}")
                for c in range(NCH8)]
    for c in range(NCH8):   # t=0 column slice first: compute starts early
        nc.sync.dma_start(
            sb["x8"][c][:, :, 0:512], dram["x8"][c, :, :, 0:512])
    for c in range(NCH8):
        t = p_w.tile([128, 2, FL], FP8, tag="w", name=f"{R}wk8{c}")
        nc.gpsimd.dma_start(t[:], dram["wk8"][c])
        sb["wk8"].append(t)
    sin_sb = p_sin.tile([128, S], BF16, tag="sin", name=f"{R}sin_sb")
    nc.gpsimd.dma_start(sin_sb[:], dram["sinq"])
    for c in range(2):
        nc.sync.dma_start(
            sb["x8"][c][:, :, 512:1024], dram["x8"][c, :, :, 512:1024])
    sb["xb"] = []
    for c in range(8):
        t = p_xb.tile([128, 512], BF16, tag="xb", name=f"{R}xb{c}")
        (nc.sync if c < 4 else nc.gpsimd).dma_start(t[:], dram["xb"][c])
        sb["xb"].append(t)
    sb["wvb"] = []
    for c in range(8):
        t = p_wb.tile([128, FL], BF16, tag="wb", name=f"{R}wvb{c}")
        (nc.sync if c < 4 else nc.gpsimd).dma_start(t[:], dram["wvb"][c])
        sb["wvb"].append(t)
    for c in range(2, NCH8):
        nc.gpsimd.dma_start(
            sb["x8"][c][:, :, 512:1024], dram["x8"][c, :, :, 512:1024])
    for c in range(NCH8):
        (nc.sync if c < 2 else nc.gpsimd).dma_start(
            sb["x8"][c][:, :, 1024:1536], dram["x8"][c, :, :, 1024:1536])
    sb["wv8"] = []
    for c in range(NCH8):
        t = p_w.tile([128, 2, FL], FP8, tag="w", name=f"{R}wv8{c}")
        nc.sync.dma_start(t[:], dram["wv8"][c])
        sb["wv8"].append(t)
    for c in range(NCH8):
        nc.sync.dma_start(
            sb["x8"][c][:, :, 1536:2048], dram["x8"][c, :, :, 1536:2048])
    sb["wo8"] = []
    for c in range(2):
        t = p_wo.tile([128, 2, D], FP8, tag="wo", name=f"{R}wo8{c}")
        nc.sync.dma_start(t[:], dram["wo8"][c])
        sb["wo8"].append(t)
    sb["wob"] = []
    for c in range(NP):
        t = p_wob.tile([128, D], BF16, tag="wob", name=f"{R}wob{c}")
        nc.sync.dma_start(t[:], dram["wob"][c])
        sb["wob"].append(t)

    # V tiles: fp8 per 256-kv chunk (+ bf16 for kv<512); ones col up front
    sb["v8"] = []
    for c in range(8):
        # 66-wide per head: DoubleRow LDWEIGHTS needs the pair-dim byte
        # step (8*66=528) to be 16-aligned; col 64 = ones, col 65 = pad
        vt = p_v.tile([128, 2, HL, 66], FP8, tag="v", name=f"{R}v8_{c}")
        nc.gpsimd.memset(vt[:, :, :, 64:65], 1.0)
        sb["v8"].append(vt)
    sb["vb"] = []
    for c in range(4):
        vt = p_vb.tile([128, HL, 65], BF16, tag="vb", name=f"{R}vb{c}")
        nc.gpsimd.memset(vt[:, :, 64:65], 1.0)
        sb["vb"].append(vt)

    # yt: bf16 [128, 512] per pair for t=0; fp8 DoubleRow layout for t>=1
    sb["ytb"] = [p_ytb.tile([128, 512], BF16, tag="ytb", name=f"{R}ytb{i}")
                 for i in range(NP)]
    sb["yt8"] = [p_yt.tile([128, 2, 1536], FP8, tag="yt", name=f"{R}yt8{i}")
                 for i in range(2)]
    qt_sb = [p_qk.tile([128, S], BF16, tag="qk", name=f"{R}qt{i}")
             for i in range(NP)]
    kt_sb = [p_qk.tile([128, S], BF16, tag="qk", name=f"{R}kt{i}")
             for i in range(NP)]

    def emit_qkproj(p, t):
        # Q and K projection slice (fp8 DoubleRow), rope fused into the drain
        for w_sb, dst, nm in ((sb["wq8"], qt_sb[p], "q"),
                              (sb["wk8"], kt_sb[p], "k")):
            ps = ps_proj.tile([128, 512], F32, tag="psp",
                              name=f"{R}{nm}ps{p}{t}")
            for c in range(NCH8):
                nc.tensor.matmul(
                    ps[:],
                    w_sb[c][:, :, 128 * p:128 * (p + 1)],
                    sb["x8"][c][:, :, 512 * t:512 * (t + 1)],
                    perf_mode=DR,
                    start=(c == 0), stop=(c == NCH8 - 1))
            if p == 0 and t == 0 and nm == "k":
                # cold start: the first score chunk only reads kt[:, 0:128];
                # a split drain lets it go ~0.5us earlier
                nc.vector.tensor_tensor(
                    out=dst[:, 0:128], in0=ps[:, 0:128],
                    in1=sin_sb[:, 0:128], op=mult)
                nc.vector.tensor_tensor(
                    out=dst[:, 128:512], in0=ps[:, 128:512],
                    in1=sin_sb[:, 128:512], op=mult)
            else:
                nc.vector.tensor_tensor(
                    out=dst[:, 512 * t:512 * (t + 1)],
                    in0=ps[:], in1=sin_sb[:, 512 * t:512 * (t + 1)],
                    op=mult)

    emit_qkproj(0, 0)
    # attention for heads 2p (e=0) and 2p+1 (e=1). Scores for both parities
    # share one [128, 2, 512] S^T psum tile; their K=64 matmuls pack PE
    # row-groups 0-1/2-3 concurrently, and one exp covers both parities.
    # P@V runs one chunk behind the S matmuls so PE never waits on ACT.
    # Block order: pairs {0,1} interleaved over t first — two pairs' early
    # exp work covers the DMA window for the later x slices — then p-outer
    # for {2,3} (long blocks keep the psum-pool recycling off the exp path).
    SEQ = ([(0, 0), (0, 1), (1, 0), (1, 1), (0, 2), (1, 2), (0, 3), (1, 3)]
           + [(p, t) for p in (2, 3) for t in range(NSQ)])

    def emit_pv0(pp, o_ps_l, pts_l):
        for cp, offp, ptp in pts_l:
            for e in range(2):
                nc.tensor.matmul(
                    o_ps_l[e][:, offp:512], sb["vb"][cp][:, 2 * pp + e, :],
                    ptp[:, e, offp:512],
                    start=(cp == 0), stop=(cp == 3))

    def emit_norm(pp, tt, o_ps_l):
        # recip of the ones-row, broadcast, scale the 64 rows. Only the
        # last pair's last tile runs in 256-wide halves (it alone gates the
        # tail O-proj); everything else full-width.
        halves = (((0, 256), (256, 512))
                  if (tt == NSQ - 1 and pp == NP - 1) else ((0, 512),))
        for h0, h1 in halves:
            for e in range(2):
                recip = p_r.tile([1, h1 - h0], F32, tag="rc",
                                 name=f"{R}rc{pp}_{tt}{e}_{h0}")
                nc.vector.reciprocal(recip[:], o_ps_l[e][64:65, h0:h1])
                rb = p_r.tile([64, h1 - h0], F32, tag="rb",
                              name=f"{R}rb{pp}_{tt}{e}_{h0}")
                nc.gpsimd.partition_broadcast(rb[:], recip[:], channels=64)
                if tt == 0:
                    dst = sb["ytb"][pp][64 * e:64 * (e + 1), h0:h1]
                else:
                    dst = sb["yt8"][pp // 2][64 * e:64 * (e + 1), pp % 2,
                                             512 * tt - 512 + h0:
                                             512 * tt - 512 + h1]
                nc.vector.tensor_tensor(
                    out=dst, in0=o_ps_l[e][0:64, h0:h1], in1=rb[:], op=mult)

    def emit_oproj_unit(q, o, tt):
        # tail units alternate through the pso pool (idle after the last
        # norm) so psp frees early for the next rep's first projections
        pool = ps_o if (tt == NSQ - 1 and o == 1) else ps_proj
        ps = pool.tile([128, 512], F32, tag="psp" if pool is ps_proj else "pso",
                       name=f"{R}ops{q}{o}")
        if tt == 0:
            for pp in range(NP):
                nc.tensor.matmul(
                    ps[:],
                    sb["ytb"][pp][:, 128 * q:128 * (q + 1)],
                    sb["wob"][pp][:, 512 * o:512 * (o + 1)],
                    start=(pp == 0), stop=(pp == NP - 1))
        else:
            for c2 in range(2):
                nc.tensor.matmul(
                    ps[:],
                    sb["yt8"][c2][:, :, 128 * q - 512:128 * (q + 1) - 512],
                    sb["wo8"][c2][:, :, 512 * o:512 * (o + 1)],
                    perf_mode=DR, start=(c2 == 0), stop=(c2 == 1))
        os_t = p_os.tile([128, 512], F32, tag="os", name=f"{R}os{q}{o}")
        if tt == NSQ - 1:
            # tail: ACT idles after the last exp and DVE after the last
            # yt; split the drains. Pool's DMA queue has no selects left.
            if o == 0:
                nc.scalar.mul(os_t[:], ps[:], 1.0 / 128.0)
            else:
                nc.vector.tensor_scalar_mul(os_t[:], ps[:], 1.0 / 128.0)
            eng = nc.sync if o == 0 else nc.gpsimd
        else:
            nc.vector.tensor_scalar_mul(os_t[:], ps[:], 1.0 / 128.0)
            eng = nc.sync
        eng.dma_start(
            dram["out"][128 * q:128 * (q + 1), 512 * o:512 * (o + 1)],
            os_t[:])

    def emit_pv8_flush(pp, tt, o_ps_l, ptt, pvq):
        n = len(pvq)
        for idx, (kind, cc2, _) in enumerate(pvq):
            first, last = idx == 0, idx == n - 1
            if kind == "full":
                ptp = ptt[("f", cc2)]
                for e in range(2):
                    nc.tensor.matmul(
                        o_ps_l[e][:, :], sb["v8"][cc2][:, :, 2 * pp + e, 0:65],
                        ptp[:, :, e, :], perf_mode=DR, start=first, stop=last)
            else:
                c = cc2
                off = 128 * (c - 4 * tt)
                ptp = ptt[("f", c // 2)]
                for e in range(2):
                    nc.tensor.matmul(
                        o_ps_l[e][:, off:512],
                        sb["v8"][c // 2][:, c % 2, 2 * pp + e, 0:65],
                        ptp[:, c % 2, e, off:512], start=first, stop=last)

    hold00 = []
    hold01 = []
    oproj_pending = []
    for bi, (p, t) in enumerate(SEQ):
        if True:
            nxt = SEQ[bi + 1] if bi + 1 < len(SEQ) else None
            o_ps = [ps_o.tile([65, 512], F32, tag="pso",
                              name=f"{R}o{p}_{t}_{i}") for i in range(2)]

            if t == 0:
                # bf16 path (4 diagonal 128-chunks). vproj/PV emitted after
                # the score/exp stream: their xb/wvb DMAs land late and must
                # not clog the PE wait queue ahead of the scores. Pair 0's
                # PV+norm are deferred into pair 1's block for the same
                # reason.
                pts = []
                for c in range(4):
                    off = 128 * c
                    w = 512 - off
                    s_ps = ps_s.tile([128, 2, 512], F32, tag="s",
                                     name=f"{R}s{p}_0_{c}")
                    for e in range(2):
                        nc.tensor.matmul(
                            s_ps[:, e, 0:w],
                            kt_sb[p][64 * e:64 * (e + 1),
                                     128 * c:128 * (c + 1)],
                            qt_sb[p][64 * e:64 * (e + 1), off:512],
                            start=True, stop=True)
                    if c == 1 and nxt is not None:
                        emit_qkproj(*nxt)
                    pt = p_ptb.tile([128, 2, 512], BF16, tag="ptb",
                                    name=f"{R}pt{p}_0_{c}")
                    if off:
                        nc.scalar.activation(
                            pt[:, :, off:512], s_ps[:, :, 0:w],
                            EXP, scale=EXP_SCALE)
                    else:
                        nc.scalar.activation(
                            pt[:].rearrange("p a b -> p (a b)"),
                            s_ps[:].rearrange("p a b -> p (a b)"),
                            EXP, scale=EXP_SCALE)
                    nc.gpsimd.affine_select(
                        out=pt[:, :, off:off + 128],
                        in_=pt[:, :, off:off + 128],
                        compare_op=is_ge, fill=0.0, base=0,
                        pattern=[[0, 2], [1, 128]], channel_multiplier=-1)
                    pts.append((c, off, pt))
                if p == 0:
                    hold00.append((o_ps, pts))
                    continue
                if p == 1 and hold00:
                    # the full deferred train: vb/v8 projections (late DMA)
                    # plus pair 0's t0 and t1 PV+norm, all emitted behind
                    # this block's exp stream so no block's scores ever sit
                    # behind it in the PE queue
                    for q in range(8):
                        _emit_vproj(nc, P, sb, q, rep)
                    o_ps00, pts00 = hold00.pop()
                    emit_pv0(0, o_ps00, pts00)
                    emit_norm(0, 0, o_ps00)
                    o_ps01, ptt01, pvq01 = hold01.pop()
                    emit_pv8_flush(0, 1, o_ps01, ptt01, pvq01)
                    emit_norm(0, 1, o_ps01)
                emit_pv0(p, o_ps, pts)
            else:
                # fp8 path: full 256-chunks DoubleRow, diagonal 128-chunks
                # plain fp8. pt8 per 256-chunk: [128, 2(kv-half), 2(par), 512].
                pt_tiles = {}
                pv_queue = []   # (kind, c2_or_c, gate_cc)
                for c2 in range(2 * t):
                    pv_queue.append(("full", c2, 2 * c2 + 1))
                for c in range(4 * t, 4 * (t + 1)):
                    pv_queue.append(("diag", c, c))
                n_pv = len(pv_queue)
                emitted = [0]

                def emit_pv8(stop_at):
                    while emitted[0] < stop_at:
                        kind, cc2, _ = pv_queue[emitted[0]]
                        first = emitted[0] == 0
                        last = emitted[0] == n_pv - 1
                        if kind == "full":
                            ptp = pt_tiles[("f", cc2)]
                            for e in range(2):
                                nc.tensor.matmul(
                                    o_ps[e][:, :],
                                    sb["v8"][cc2][:, :, 2 * p + e, 0:65],
                                    ptp[:, :, e, :], perf_mode=DR,
                                    start=first, stop=last)
                        else:
                            c = cc2
                            off = 128 * (c - 4 * t)
                            ptp = pt_tiles[("f", c // 2)]
                            for e in range(2):
                                nc.tensor.matmul(
                                    o_ps[e][:, off:512],
                                    sb["v8"][c // 2][:, c % 2, 2 * p + e, 0:65],
                                    ptp[:, c % 2, e, off:512],
                                    start=first, stop=last)
                        emitted[0] += 1

                for cc in range(4 * (t + 1)):
                    c2, i = cc // 2, cc % 2
                    dc = cc - 4 * t
                    off = 128 * dc if dc > 0 else 0
                    w = 512 - off
                    s_ps = ps_s.tile([128, 2, 512], F32, tag="s",
                                     name=f"{R}s{p}_{t}_{cc}")
                    for e in range(2):
                        nc.tensor.matmul(
                            s_ps[:, e, 0:w],
                            kt_sb[p][64 * e:64 * (e + 1),
                                     128 * cc:128 * (cc + 1)],
                            qt_sb[p][64 * e:64 * (e + 1),
                                     512 * t + off:512 * (t + 1)],
                            start=True, stop=True)
                    if p == 0 and cc < 4 and bi != 1:
                        _emit_vproj(nc, P, sb, 4 * t + cc, rep)
                    if cc == 1 and nxt is not None:
                        emit_qkproj(*nxt)
                    if ("f", c2) not in pt_tiles:
                        pt_tiles[("f", c2)] = p_pt.tile(
                            [128, 2, 2, 512], FP8, tag="pt",
                            name=f"{R}pt{p}_{t}_{c2}")
                    pt = pt_tiles[("f", c2)]
                    if off:
                        nc.scalar.activation(
                            pt[:, i, :, off:512], s_ps[:, :, 0:w],
                            EXP, scale=EXP_SCALE)
                    else:
                        nc.scalar.activation(
                            pt[:, i, :, :].rearrange("p a b -> p (a b)"),
                            s_ps[:].rearrange("p a b -> p (a b)"),
                            EXP, scale=EXP_SCALE)
                    if dc >= 0:
                        # causal boundary within [off:off+128]
                        nc.gpsimd.affine_select(
                            out=pt[:, i, :, off:off + 128],
                            in_=pt[:, i, :, off:off + 128],
                            compare_op=is_ge, fill=0.0, base=0,
                            pattern=[[0, 2], [1, 128]], channel_multiplier=-1)
                    # emit PV units whose pt is complete, one chunk behind
                    if bi != 1:
                        ready = sum(1 for u in pv_queue if u[2] <= cc - 1)
                        emit_pv8(ready)
                    if oproj_pending:
                        emit_oproj_unit(*oproj_pending.pop(0))
                if bi == 1:
                    # stash: this block's PV depends on the vproj train
                    # (late DMA); both are deferred into block (1,0)
                    hold01.append((o_ps, pt_tiles, pv_queue))
                else:
                    emit_pv8(n_pv)

            if bi == 1:
                continue
            emit_norm(p, t, o_ps)

            if p == NP - 1:
                # output projection for the seq tiles this t completes.
                # t<3 units are queued and interleaved into the NEXT block's
                # chunk loop so their matmuls/drains don't sit ahead of that
                # block's scores in the PE/DVE streams; t=3 is the tail.
                units = [(q, o, t) for q in range(4 * t, 4 * (t + 1))
                         for o in range(2)]
                if t < NSQ - 1:
                    oproj_pending.extend(units)
                else:
                    while oproj_pending:
                        emit_oproj_unit(*oproj_pending.pop(0))
                    for u in units:
                        emit_oproj_unit(*u)


def build_nc(reps=1):
    key = ("nc", reps)
    if key in _CACHED:
        return _CACHED[key]
    from contextlib import ExitStack

    # Honest ACT per-instruction overhead for the tile scheduler's cost
    # model (measured ~352 cycles vs the default 172/222): the static
    # per-engine order then interleaves projection matmuls into the
    # ACT-gated attention stretches instead of stalling PE on exp.
    try:
        from concourse.hw_specs import TRN2Spec
        from concourse.bass import MemorySpace
        TRN2Spec.ACCESS_CYCLES[(MemorySpace.PSUM, mybir.EngineType.Activation)] = 352
        TRN2Spec.ACCESS_CYCLES[(MemorySpace.SBUF, mybir.EngineType.Activation)] = 352
    except Exception:
        pass

    nc = bacc.Bacc("TRN2", target_bir_lowering=False, debug=False,
                   num_devices=NCORES)
    dram = {
        "x8": nc.dram_tensor("x8", [NCH8, 128, 2, S], FP8,
                             kind="ExternalInput").ap(),
        "xb": nc.dram_tensor("xb", [8, 128, 512], BF16,
                             kind="ExternalInput").ap(),
        "wq8": nc.dram_tensor("wq8", [NCH8, 128, 2, FL], FP8,
                              kind="ExternalInput").ap(),
        "wk8": nc.dram_tensor("wk8", [NCH8, 128, 2, FL], FP8,
                              kind="ExternalInput").ap(),
        "wv8": nc.dram_tensor("wv8", [NCH8, 128, 2, FL], FP8,
                              kind="ExternalInput").ap(),
        "wvb": nc.dram_tensor("wvb", [8, 128, FL], BF16,
                              kind="ExternalInput").ap(),
        "wo8": nc.dram_tensor("wo8", [2, 128, 2, D], FP8,
                              kind="ExternalInput").ap(),
        "wob": nc.dram_tensor("wob", [NP, 128, D], BF16,
                              kind="ExternalInput").ap(),
        "sinq": nc.dram_tensor("sinq", [128, S], BF16,
                               kind="ExternalInput").ap(),
        "out": nc.dram_tensor("out", [S, D], F32, kind="ExternalOutput").ap(),
    }

    import os
    trace_sim = bool(os.environ.get("KTRACE"))
    with tile.TileContext(nc, trace_sim=trace_sim) as tc, ExitStack() as ctx:
        P = (
            ctx.enter_context(tc.tile_pool(name="x", bufs=2 * NCH8)),
            ctx.enter_context(tc.tile_pool(name="xb", bufs=8)),
            ctx.enter_context(tc.tile_pool(name="w", bufs=3 * NCH8 + 4)),
            ctx.enter_context(tc.tile_pool(name="wb", bufs=8)),
            ctx.enter_context(tc.tile_pool(name="wo", bufs=2)),
            ctx.enter_context(tc.tile_pool(name="wob", bufs=NP)),
            ctx.enter_context(tc.tile_pool(name="qk", bufs=2 * NP)),
            ctx.enter_context(tc.tile_pool(name="v", bufs=8)),
            ctx.enter_context(tc.tile_pool(name="vb", bufs=4)),
            ctx.enter_context(tc.tile_pool(name="sin", bufs=2)),
            ctx.enter_context(tc.tile_pool(name="pt", bufs=8)),
            ctx.enter_context(tc.tile_pool(name="ptb", bufs=8)),
            ctx.enter_context(tc.tile_pool(name="yt", bufs=2)),
            ctx.enter_context(tc.tile_pool(name="ytb", bufs=NP)),
            ctx.enter_context(tc.tile_pool(name="r", bufs=4)),
            ctx.enter_context(tc.tile_pool(name="os", bufs=4)),
            ctx.enter_context(tc.tile_pool(name="psp", bufs=2, space="PSUM")),
            ctx.enter_context(tc.tile_pool(name="pss", bufs=2, space="PSUM")),
            ctx.enter_context(tc.tile_pool(name="pso", bufs=2, space="PSUM")),
        )
        for rep in range(reps):
            _emit_body(nc, P, dram, rep)

    nc.finalize()
    _CACHED[key] = nc
    return nc


def _host_prep(x, wq, wk, wv, wo):
    """Fold RoPE rotation + fp8 scale into the weights; slice per core."""
    rope_dim = AOD // 2
    j = np.arange(rope_dim, dtype=np.float32)
    thetas = (1.0 / (10000.0 ** (2.0 * j / rope_dim))).astype(np.float32)
    pos = np.arange(S, dtype=np.float32)
    angles = pos[:, None] * thetas[None, :]          # [S, 32]
    sinv = np.sin(angles).astype(np.float32)         # [S, 32]
    # sin pattern tile [128, S]: row r multiplies feature (64*pair + r%64);
    # /16 descales the x128 fp8 weight scaling down to qt = 8*q
    sin2 = (np.tile(sinv.T, (4, 1)) / 16.0).astype(np.float32)

    def fold(w):
        wr = w.reshape(D, H, 2, rope_dim)
        return np.concatenate(
            [wr[:, :, 0] - wr[:, :, 1], wr[:, :, 0] + wr[:, :, 1]],
            axis=2).reshape(D, D)

    wqf = fold(wq) * 128.0
    wkf = fold(wk) * 128.0

    bf = ml_dtypes.bfloat16
    f8 = ml_dtypes.float8_e4m3

    def dr(w):  # [D, M] -> [NCH8, 128, 2, M] DoubleRow layout, fp8
        m = w.shape[1]
        return np.ascontiguousarray(
            w.reshape(NCH8, 2, 128, m).transpose(0, 2, 1, 3)).astype(f8)

    in_maps = []
    for core in range(NCORES):
        b, g = divmod(core, 2)
        sl = slice(g * FL, (g + 1) * FL)
        xT = np.ascontiguousarray(x[b].T)            # [D, S] f32
        wo_l = wo[sl, :] * 8.0                       # [FL, D]
        in_maps.append({
            "x8": dr(xT),
            "xb": np.ascontiguousarray(
                xT[:, 0:512].reshape(8, 128, 512)).astype(bf),
            "wq8": dr(wqf[:, sl]),
            "wk8": dr(wkf[:, sl]),
            "wv8": dr(wv[:, sl] * 128.0),
            "wvb": np.ascontiguousarray(
                (wv[:, sl] * 128.0).reshape(8, 128, FL)).astype(bf),
            "wo8": np.ascontiguousarray(
                wo_l.reshape(2, 2, 128, D).transpose(0, 2, 1, 3)).astype(f8),
            "wob": np.ascontiguousarray(
                wo_l.reshape(NP, 128, D)).astype(bf),
            "sinq": sin2.astype(bf),
        })
    return in_maps


def kernel(x, wq, wk, wv, wo, bo):
    nc = build_nc()
    in_maps = _host_prep(np.asarray(x, np.float32), np.asarray(wq, np.float32),
                         np.asarray(wk, np.float32), np.asarray(wv, np.float32),
                         np.asarray(wo, np.float32))
    res = run_bass_kernel_spmd(nc, in_maps, list(range(NCORES)))
    out = np.empty((B, S, D), np.float32)
    bo32 = np.asarray(bo, np.float32)
    for b in range(B):
        out[b] = res.results[2 * b]["out"] + res.results[2 * b + 1]["out"] + bo32
    return out


# revision 50
# speedup vs baseline: 1.8736x; 1.0804x over previous
"""Causal multi-head attention with (buggy-faithful) RoPE on 8 trn2 cores.

Problem: B=4, S=2048, D=1024, H=16 heads of dim 64, fp32.
Sharding: core c handles batch b=c//2 and head-group g=c%2 (8 heads).
Each core computes partial_out = attn(x_b, heads g) @ wo[rows g]; the host
sums the two partials per batch and adds the bias.

v2: fp8 (e4m3) DoubleRow matmuls for the PE-heavy stages. Double FP8 packs
two contraction rows per partition (K_eff=256) for 2x tensor-engine
throughput. Precision plan (validated host-side, rel err ~2e-3):
- Q/K projections: fp8 DoubleRow (q/k noise ~4% is washed out by softmax).
- V projection: bf16 for kv<512, fp8 DoubleRow for kv>=512.
- P@V + the attention probs pt: fp8 for q>=512 (t>=1), bf16 for t=0. Early
  rows have peaked attention (y ~= v_k), so V/pt quantization error there
  lands directly in the output max; rows>=512 average over >=512 positions.
- Output projection: fp8 DoubleRow for q>=512, bf16 for t=0.
- Scores stay bf16 (K=64 per head; two heads pack PE row-groups 0-1/2-3
  concurrently via tile_position, so bf16 scores already run at full rate).

Scale folding keeps every fp8 tensor in e4m3's happy range (max 240):
wq/wk *128 (host) with /16 folded into the bf16 sin table -> qt,kt = 8*q,8*k
(exp scale 0.125/64); wv *128 with *1/8 in the drain -> v16 = 16*v; ones
column 1.0 -> yt = 16*y; wo *8 -> out psum = 128*out, *1/128 in the drain.

On-device layout (per core):
- x8 [4][128, 2, S] fp8: feature d = 256c+128i+p (DoubleRow pair on i).
- QT/KT [512, 2048] bf16 feature-major; V as [128, 2, 8, 65] fp8 per 256-kv
  chunk (65th col = ones -> softmax denominator accumulates in the P@V
  matmul) + bf16 [128, 8, 65] for kv<512.
- S^T [kv, q] per head; exp needs no max subtraction (|scores/8| < ~3).
- pt8 [128, 2, 2, 512] fp8 per 256-kv chunk: (kv-half i, parity e, q).
- P@V full 256-chunks: one DoubleRow matmul per head; diagonal 128-chunks:
  plain fp8 matmuls over the trapezoid, mask boundary via affine_select.
"""

import numpy as np
import ml_dtypes

import concourse.bacc as bacc
import concourse.mybir as mybir
import concourse.tile as tile
from concourse.bass_utils import run_bass_kernel_spmd

B, S, D = 4, 2048, 1024
H = 16
AOD = 64
HL = 8            # heads per core
FL = HL * AOD     # 512 local features
NCORES = 8
NCH8 = 4          # fp8 DoubleRow contraction chunks (256 features each)
NSQ = 4           # sq tiles of 512
NP = FL // 128    # 4 feature chunks (head pairs)
NT16 = S // 128   # 16 seq chunks of 128

F32 = mybir.dt.float32
BF16 = mybir.dt.bfloat16
FP8 = mybir.dt.float8e4
EXP = mybir.ActivationFunctionType.Exp
DR = mybir.MatmulPerfMode.DoubleRow
EXP_SCALE = 0.125 / 64.0   # qt,kt carry x8 each

_CACHED = {}


def _alu():
    from concourse.alu_op_type import AluOpType
    return AluOpType


def _emit_vproj(nc, P, sb, q, rep):
    """V projection for seq tile q (128 positions): writes v8 (fp8, 16*v)
    and, for q<4, also vb (bf16, 16*v) for the t=0 path."""
    (p_x, p_xb, p_w, p_wb, p_wo, p_wob, p_qk, p_v, p_vb, p_sin, p_pt,
     p_ptb, p_yt, p_ytb, p_r, p_os, ps_proj, ps_s, ps_o) = P
    ps = ps_proj.tile([128, FL], F32, tag="psp", name=f"r{rep}vps{q}")
    if q < 4:
        for c in range(8):
            nc.tensor.matmul(
                ps[:], sb["xb"][c][:, 128 * q:128 * (q + 1)], sb["wvb"][c][:],
                start=(c == 0), stop=(c == 7))
    else:
        for c in range(NCH8):
            nc.tensor.matmul(
                ps[:], sb["x8"][c][:, :, 128 * q:128 * (q + 1)],
                sb["wv8"][c][:], perf_mode=DR,
                start=(c == 0), stop=(c == NCH8 - 1))
    nc.vector.tensor_scalar_mul(
        sb["v8"][q // 2][:, q % 2, :, 0:64],
        ps[:].rearrange("p (h d) -> p h d", h=HL), 0.125)
    if q < 4:
        nc.vector.tensor_scalar_mul(
            sb["vb"][q][:, :, 0:64],
            ps[:].rearrange("p (h d) -> p h d", h=HL), 0.125)


def _emit_body(nc, P, dram, rep):
    """One full forward pass for this core's shard."""
    mult = _alu().mult
    is_ge = _alu().is_ge
    (p_x, p_xb, p_w, p_wb, p_wo, p_wob, p_qk, p_v, p_vb, p_sin, p_pt,
     p_ptb, p_yt, p_ytb, p_r, p_os, ps_proj, ps_s, ps_o) = P
    R = f"r{rep}"

    # ---- resident loads, split across the qSP and qPool DMA queues so the
    # first-needed tensors land in ~3us instead of ~17us serial. The Pool
    # queue only carries the early small loads: gpsimd DMA blocks the Pool
    # engine for the transfer, and Pool must stay free for the causal-mask
    # affine_selects in the exp->PV chain. ----
    sb = {}
    # warm the ACT exp table during the DMA window (saves ~1.4us at first
    # exp); the table persists, so only the first rep needs it
    if rep == 0:
        warm = p_r.tile([1, 16], F32, tag="warm", name=f"{R}warm")
        nc.gpsimd.memset(warm[:], 0.0)
        nc.scalar.activation(warm[:], warm[:], EXP, scale=1.0)
    sb["wq8"] = []
    sb["wk8"] = []
    for c in range(NCH8):
        t = p_w.tile([128, 2, FL], FP8, tag="w", name=f"{R}wq8{c}")
        (nc.sync if c < 2 else nc.gpsimd).dma_start(t[:], dram["wq8"][c])
        sb["wq8"].append(t)
    sb["x8"] = [p_x.tile([128, 2, S], FP8, tag="x", name=f"{R}x{c}")
                for c in range(NCH8)]
    for c in range(NCH8):   # t=0 column slice first: compute starts early
        nc.sync.dma_start(
            sb["x8"][c][:, :, 0:512], dram["x8"][c, :, :, 0:512])
    for c in range(NCH8):
        t = p_w.tile([128, 2, FL], FP8, tag="w", name=f"{R}wk8{c}")
        nc.gpsimd.dma_start(t[:], dram["wk8"][c])
        sb["wk8"].append(t)
    sin_sb = p_sin.tile([128, S], BF16, tag="sin", name=f"{R}sin_sb")
    nc.gpsimd.dma_start(sin_sb[:], dram["sinq"])
    for c in range(2):
        nc.sync.dma_start(
            sb["x8"][c][:, :, 512:1024], dram["x8"][c, :, :, 512:1024])
    sb["xb"] = []
    for c in range(8):
        t = p_xb.tile([128, 512], BF16, tag="xb", name=f"{R}xb{c}")
        (nc.sync if c < 4 else nc.gpsimd).dma_start(t[:], dram["xb"][c])
        sb["xb"].append(t)
    sb["wvb"] = []
    for c in range(8):
        t = p_wb.tile([128, FL], BF16, tag="wb", name=f"{R}wvb{c}")
        (nc.sync if c < 4 else nc.gpsimd).dma_start(t[:], dram["wvb"][c])
        sb["wvb"].append(t)
    for c in range(2, NCH8):
        nc.gpsimd.dma_start(
            sb["x8"][c][:, :, 512:1024], dram["x8"][c, :, :, 512:1024])
    for c in range(NCH8):
        (nc.sync if c < 2 else nc.gpsimd).dma_start(
            sb["x8"][c][:, :, 1024:1536], dram["x8"][c, :, :, 1024:1536])
    sb["wv8"] = []
    for c in range(NCH8):
        t = p_w.tile([128, 2, FL], FP8, tag="w", name=f"{R}wv8{c}")
        nc.sync.dma_start(t[:], dram["wv8"][c])
        sb["wv8"].append(t)
    for c in range(NCH8):
        nc.sync.dma_start(
            sb["x8"][c][:, :, 1536:2048], dram["x8"][c, :, :, 1536:2048])
    sb["wo8"] = []
    for c in range(2):
        t = p_wo.tile([128, 2, D], FP8, tag="wo", name=f"{R}wo8{c}")
        nc.sync.dma_start(t[:], dram["wo8"][c])
        sb["wo8"].append(t)
    sb["wob"] = []
    for c in range(NP):
        t = p_wob.tile([128, D], BF16, tag="wob", name=f"{R}wob{c}")
        nc.sync.dma_start(t[:], dram["wob"][c])
        sb["wob"].append(t)

    # V tiles: fp8 per 256-kv chunk (+ bf16 for kv<512); ones col up front
    sb["v8"] = []
    for c in range(8):
        # 66-wide per head: DoubleRow LDWEIGHTS needs the pair-dim byte
        # step (8*66=528) to be 16-aligned; col 64 = ones, col 65 = pad
        vt = p_v.tile([128, 2, HL, 66], FP8, tag="v", name=f"{R}v8_{c}")
        nc.gpsimd.memset(vt[:, :, :, 64:65], 1.0)
        sb["v8"].append(vt)
    sb["vb"] = []
    for c in range(4):
        vt = p_vb.tile([128, HL, 65], BF16, tag="vb", name=f"{R}vb{c}")
        nc.gpsimd.memset(vt[:, :, 64:65], 1.0)
        sb["vb"].append(vt)

    # yt: bf16 [128, 512] per pair for t=0; fp8 DoubleRow layout for t>=1
    sb["ytb"] = [p_ytb.tile([128, 512], BF16, tag="ytb", name=f"{R}ytb{i}")
                 for i in range(NP)]
    sb["yt8"] = [p_yt.tile([128, 2, 1536], FP8, tag="yt", name=f"{R}yt8{i}")
                 for i in range(2)]
    qt_sb = [p_qk.tile([128, S], BF16, tag="qk", name=f"{R}qt{i}")
             for i in range(NP)]
    kt_sb = [p_qk.tile([128, S], BF16, tag="qk", name=f"{R}kt{i}")
             for i in range(NP)]

    def emit_qkproj(p, t):
        # Q and K projection slice (fp8 DoubleRow), rope fused into the drain
        for w_sb, dst, nm in ((sb["wq8"], qt_sb[p], "q"),
                              (sb["wk8"], kt_sb[p], "k")):
            ps = ps_proj.tile([128, 512], F32, tag="psp",
                              name=f"{R}{nm}ps{p}{t}")
            for c in range(NCH8):
                nc.tensor.matmul(
                    ps[:],
                    w_sb[c][:, :, 128 * p:128 * (p + 1)],
                    sb["x8"][c][:, :, 512 * t:512 * (t + 1)],
                    perf_mode=DR,
                    start=(c == 0), stop=(c == NCH8 - 1))
            if p == 0 and t == 0 and nm == "k":
                # cold start: the first score chunk only reads kt[:, 0:128];
                # a split drain lets it go ~0.5us earlier
                nc.vector.tensor_tensor(
                    out=dst[:, 0:128], in0=ps[:, 0:128],
                    in1=sin_sb[:, 0:128], op=mult)
                nc.vector.tensor_tensor(
                    out=dst[:, 128:512], in0=ps[:, 128:512],
                    in1=sin_sb[:, 128:512], op=mult)
            else:
                nc.vector.tensor_tensor(
                    out=dst[:, 512 * t:512 * (t + 1)],
                    in0=ps[:], in1=sin_sb[:, 512 * t:512 * (t + 1)],
                    op=mult)

    emit_qkproj(0, 0)
    # attention for heads 2p (e=0) and 2p+1 (e=1). Scores for both parities
    # share one [128, 2, 512] S^T psum tile; their K=64 matmuls pack PE
    # row-groups 0-1/2-3 concurrently, and one exp covers both parities.
    # P@V runs one chunk behind the S matmuls so PE never waits on ACT.
    # Block order: pairs {0,1} interleaved over t first — two pairs' early
    # exp work covers the DMA window for the later x slices — then p-outer
    # for {2,3} (long blocks keep the psum-pool recycling off the exp path).
    SEQ = ([(0, 0), (0, 1), (1, 0), (1, 1), (0, 2), (1, 2), (0, 3), (1, 3)]
           + [(p, t) for p in (2, 3) for t in range(NSQ)])

    def emit_pv0(pp, o_ps_l, pts_l):
        for cp, offp, ptp in pts_l:
            for e in range(2):
                nc.tensor.matmul(
                    o_ps_l[e][:, offp:512], sb["vb"][cp][:, 2 * pp + e, :],
                    ptp[:, e, offp:512],
                    start=(cp == 0), stop=(cp == 3))

    def emit_norm(pp, tt, o_ps_l):
        # recip of the ones-row, broadcast, scale the 64 rows. Only the
        # last pair's last tile runs in 256-wide halves (it alone gates the
        # tail O-proj); everything else full-width.
        halves = (((0, 256), (256, 512))
                  if (tt == NSQ - 1 and pp == NP - 1) else ((0, 512),))
        for h0, h1 in halves:
            for e in range(2):
                recip = p_r.tile([1, h1 - h0], F32, tag="rc",
                                 name=f"{R}rc{pp}_{tt}{e}_{h0}")
                nc.vector.reciprocal(recip[:], o_ps_l[e][64:65, h0:h1])
                rb = p_r.tile([64, h1 - h0], F32, tag="rb",
                              name=f"{R}rb{pp}_{tt}{e}_{h0}")
                nc.gpsimd.partition_broadcast(rb[:], recip[:], channels=64)
                if tt == 0:
                    dst = sb["ytb"][pp][64 * e:64 * (e + 1), h0:h1]
                else:
                    dst = sb["yt8"][pp // 2][64 * e:64 * (e + 1), pp % 2,
                                             512 * tt - 512 + h0:
                                             512 * tt - 512 + h1]
                nc.vector.tensor_tensor(
                    out=dst, in0=o_ps_l[e][0:64, h0:h1], in1=rb[:], op=mult)

    def emit_oproj_unit(q, o, tt):
        # tail units alternate through the pso pool (idle after the last
        # norm) so psp frees early for the next rep's first projections
        pool = ps_o if (tt == NSQ - 1 and o == 1) else ps_proj
        ps = pool.tile([128, 512], F32, tag="psp" if pool is ps_proj else "pso",
                       name=f"{R}ops{q}{o}")
        if tt == 0:
            for pp in range(NP):
                nc.tensor.matmul(
                    ps[:],
                    sb["ytb"][pp][:, 128 * q:128 * (q + 1)],
                    sb["wob"][pp][:, 512 * o:512 * (o + 1)],
                    start=(pp == 0), stop=(pp == NP - 1))
        else:
            for c2 in range(2):
                nc.tensor.matmul(
                    ps[:],
                    sb["yt8"][c2][:, :, 128 * q - 512:128 * (q + 1) - 512],
                    sb["wo8"][c2][:, :, 512 * o:512 * (o + 1)],
                    perf_mode=DR, start=(c2 == 0), stop=(c2 == 1))
        os_t = p_os.tile([128, 512], F32, tag="os", name=f"{R}os{q}{o}")
        if tt == NSQ - 1:
            # tail: ACT idles after the last exp and DVE after the last
            # yt; split the drains. Pool's DMA queue has no selects left.
            if o == 0:
                nc.scalar.mul(os_t[:], ps[:], 1.0 / 128.0)
            else:
                nc.vector.tensor_scalar_mul(os_t[:], ps[:], 1.0 / 128.0)
            eng = nc.sync if o == 0 else nc.gpsimd
        else:
            nc.vector.tensor_scalar_mul(os_t[:], ps[:], 1.0 / 128.0)
            eng = nc.sync
        eng.dma_start(
            dram["out"][128 * q:128 * (q + 1), 512 * o:512 * (o + 1)],
            os_t[:])

    def emit_pv8_flush(pp, tt, o_ps_l, ptt, pvq):
        n = len(pvq)
        for idx, (kind, cc2, _) in enumerate(pvq):
            first, last = idx == 0, idx == n - 1
            if kind == "full":
                ptp = ptt[("f", cc2)]
                for e in range(2):
                    nc.tensor.matmul(
                        o_ps_l[e][:, :], sb["v8"][cc2][:, :, 2 * pp + e, 0:65],
                        ptp[:, :, e, :], perf_mode=DR, start=first, stop=last)
            else:
                c = cc2
                off = 128 * (c - 4 * tt)
                ptp = ptt[("f", c // 2)]
                for e in range(2):
                    nc.tensor.matmul(
                        o_ps_l[e][:, off:512],
                        sb["v8"][c // 2][:, c % 2, 2 * pp + e, 0:65],
                        ptp[:, c % 2, e, off:512], start=first, stop=last)

    hold00 = []
    hold01 = []
    oproj_pending = []
    for bi, (p, t) in enumerate(SEQ):
        if True:
            nxt = SEQ[bi + 1] if bi + 1 < len(SEQ) else None
            o_ps = [ps_o.tile([65, 512], F32, tag="pso",
                              name=f"{R}o{p}_{t}_{i}") for i in range(2)]

            if t == 0:
                # bf16 path (4 diagonal 128-chunks). vproj/PV emitted after
                # the score/exp stream: their xb/wvb DMAs land late and must
                # not clog the PE wait queue ahead of the scores. Pair 0's
                # PV+norm are deferred into pair 1's block for the same
                # reason.
                pts = []
                for c in range(4):
                    off = 128 * c
                    w = 512 - off
                    s_ps = ps_s.tile([128, 2, 512], F32, tag="s",
                                     name=f"{R}s{p}_0_{c}")
                    for e in range(2):
                        nc.tensor.matmul(
                            s_ps[:, e, 0:w],
                            kt_sb[p][64 * e:64 * (e + 1),
                                     128 * c:128 * (c + 1)],
                            qt_sb[p][64 * e:64 * (e + 1), off:512],
                            start=True, stop=True)
                    if c == 1 and nxt is not None:
                        emit_qkproj(*nxt)
                    pt = p_ptb.tile([128, 2, 512], BF16, tag="ptb",
                                    name=f"{R}pt{p}_0_{c}")
                    if off:
                        nc.scalar.activation(
                            pt[:, :, off:512], s_ps[:, :, 0:w],
                            EXP, scale=EXP_SCALE)
                    else:
                        nc.scalar.activation(
                            pt[:].rearrange("p a b -> p (a b)"),
                            s_ps[:].rearrange("p a b -> p (a b)"),
                            EXP, scale=EXP_SCALE)
                    nc.gpsimd.affine_select(
                        out=pt[:, :, off:off + 128],
                        in_=pt[:, :, off:off + 128],
                        compare_op=is_ge, fill=0.0, base=0,
                        pattern=[[0, 2], [1, 128]], channel_multiplier=-1)
                    pts.append((c, off, pt))
                if p == 0:
                    hold00.append((o_ps, pts))
                    continue
                if p == 1 and hold00:
                    # the full deferred train: vb/v8 projections (late DMA)
                    # plus pair 0's t0 and t1 PV+norm, all emitted behind
                    # this block's exp stream so no block's scores ever sit
                    # behind it in the PE queue
                    for q in range(8):
                        _emit_vproj(nc, P, sb, q, rep)
                    o_ps00, pts00 = hold00.pop()
                    emit_pv0(0, o_ps00, pts00)
                    emit_norm(0, 0, o_ps00)
                    o_ps01, ptt01, pvq01 = hold01.pop()
                    emit_pv8_flush(0, 1, o_ps01, ptt01, pvq01)
                    emit_norm(0, 1, o_ps01)
                emit_pv0(p, o_ps, pts)
            else:
                # fp8 path: full 256-chunks DoubleRow, diagonal 128-chunks
                # plain fp8. pt8 per 256-chunk: [128, 2(kv-half), 2(par), 512].
                pt_tiles = {}
                pv_queue = []   # (kind, c2_or_c, gate_cc)
                for c2 in range(2 * t):
                    pv_queue.append(("full", c2, 2 * c2 + 1))
                for c in range(4 * t, 4 * (t + 1)):
                    pv_queue.append(("diag", c, c))
                n_pv = len(pv_queue)
                emitted = [0]

                def emit_pv8(stop_at):
                    while emitted[0] < stop_at:
                        kind, cc2, _ = pv_queue[emitted[0]]
                        first = emitted[0] == 0
                        last = emitted[0] == n_pv - 1
                        if kind == "full":
                            ptp = pt_tiles[("f", cc2)]
                            for e in range(2):
                                nc.tensor.matmul(
                                    o_ps[e][:, :],
                                    sb["v8"][cc2][:, :, 2 * p + e, 0:65],
                                    ptp[:, :, e, :], perf_mode=DR,
                                    start=first, stop=last)
                        else:
                            c = cc2
                            off = 128 * (c - 4 * t)
                            ptp = pt_tiles[("f", c // 2)]
                            for e in range(2):
                                nc.tensor.matmul(
                                    o_ps[e][:, off:512],
                                    sb["v8"][c // 2][:, c % 2, 2 * p + e, 0:65],
                                    ptp[:, c % 2, e, off:512],
                                    start=first, stop=last)
                        emitted[0] += 1

                for cc in range(4 * (t + 1)):
                    c2, i = cc // 2, cc % 2
                    dc = cc - 4 * t
                    off = 128 * dc if dc > 0 else 0
                    w = 512 - off
                    s_ps = ps_s.tile([128, 2, 512], F32, tag="s",
                                     name=f"{R}s{p}_{t}_{cc}")
                    for e in range(2):
                        nc.tensor.matmul(
                            s_ps[:, e, 0:w],
                            kt_sb[p][64 * e:64 * (e + 1),
                                     128 * cc:128 * (cc + 1)],
                            qt_sb[p][64 * e:64 * (e + 1),
                                     512 * t + off:512 * (t + 1)],
                            start=True, stop=True)
                    if p == 0 and cc < 4 and bi != 1:
                        _emit_vproj(nc, P, sb, 4 * t + cc, rep)
                    if cc == 1 and nxt is not None:
                        emit_qkproj(*nxt)
                    if ("f", c2) not in pt_tiles:
                        pt_tiles[("f", c2)] = p_pt.tile(
                            [128, 2, 2, 512], FP8, tag="pt",
                            name=f"{R}pt{p}_{t}_{c2}")
                    pt = pt_tiles[("f", c2)]
                    if off:
                        nc.scalar.activation(
                            pt[:, i, :, off:512], s_ps[:, :, 0:w],
                            EXP, scale=EXP_SCALE)
                    else:
                        nc.scalar.activation(
                            pt[:, i, :, :].rearrange("p a b -> p (a b)"),
                            s_ps[:].rearrange("p a b -> p (a b)"),
                            EXP, scale=EXP_SCALE)
                    if dc >= 0:
                        # causal boundary within [off:off+128]
                        nc.gpsimd.affine_select(
                            out=pt[:, i, :, off:off + 128],
                            in_=pt[:, i, :, off:off + 128],
                            compare_op=is_ge, fill=0.0, base=0,
                            pattern=[[0, 2], [1, 128]], channel_multiplier=-1)
                    # emit PV units whose pt is complete, one chunk behind
                    if bi != 1:
                        ready = sum(1 for u in pv_queue if u[2] <= cc - 1)
                        emit_pv8(ready)
                    if oproj_pending:
                        emit_oproj_unit(*oproj_pending.pop(0))
                if bi == 1:
                    # stash: this block's PV depends on the vproj train
                    # (late DMA); both are deferred into block (1,0)
                    hold01.append((o_ps, pt_tiles, pv_queue))
                else:
                    emit_pv8(n_pv)

            if bi == 1:
                continue
            emit_norm(p, t, o_ps)

            if p == NP - 1:
                # output projection for the seq tiles this t completes.
                # t<3 units are queued and interleaved into the NEXT block's
                # chunk loop so their matmuls/drains don't sit ahead of that
                # block's scores in the PE/DVE streams; t=3 is the tail.
                units = [(q, o, t) for q in range(4 * t, 4 * (t + 1))
                         for o in range(2)]
                if t < NSQ - 1:
                    oproj_pending.extend(units)
                else:
                    while oproj_pending:
                        emit_oproj_unit(*oproj_pending.pop(0))
                    for u in units:
                        emit_oproj_unit(*u)


def build_nc(reps=1):
    key = ("nc", reps)
    if key in _CACHED:
        return _CACHED[key]
    from contextlib import ExitStack

    # Honest ACT per-instruction overhead for the tile scheduler's cost
    # model (measured ~352 cycles vs the default 172/222): the static
    # per-engine order then interleaves projection matmuls into the
    # ACT-gated attention stretches instead of stalling PE on exp.
    try:
        from concourse.hw_specs import TRN2Spec
        from concourse.bass import MemorySpace
        TRN2Spec.ACCESS_CYCLES[(MemorySpace.PSUM, mybir.EngineType.Activation)] = 352
        TRN2Spec.ACCESS_CYCLES[(MemorySpace.SBUF, mybir.EngineType.Activation)] = 352
    except Exception:
        pass

    nc = bacc.Bacc("TRN2", target_bir_lowering=False, debug=False,
                   num_devices=NCORES)
    dram = {
        "x8": nc.dram_tensor("x8", [NCH8, 128, 2, S], FP8,
                             kind="ExternalInput").ap(),
        "xb": nc.dram_tensor("xb", [8, 128, 512], BF16,
                             kind="ExternalInput").ap(),
        "wq8": nc.dram_tensor("wq8", [NCH8, 128, 2, FL], FP8,
                              kind="ExternalInput").ap(),
        "wk8": nc.dram_tensor("wk8", [NCH8, 128, 2, FL], FP8,
                              kind="ExternalInput").ap(),
        "wv8": nc.dram_tensor("wv8", [NCH8, 128, 2, FL], FP8,
                              kind="ExternalInput").ap(),
        "wvb": nc.dram_tensor("wvb", [8, 128, FL], BF16,
                              kind="ExternalInput").ap(),
        "wo8": nc.dram_tensor("wo8", [2, 128, 2, D], FP8,
                              kind="ExternalInput").ap(),
        "wob": nc.dram_tensor("wob", [NP, 128, D], BF16,
                              kind="ExternalInput").ap(),
        "sinq": nc.dram_tensor("sinq", [128, S], BF16,
                               kind="ExternalInput").ap(),
        "out": nc.dram_tensor("out", [S, D], F32, kind="ExternalOutput").ap(),
    }

    import os
    trace_sim = bool(os.environ.get("KTRACE"))
    with tile.TileContext(nc, trace_sim=trace_sim) as tc, ExitStack() as ctx:
        P = (
            ctx.enter_context(tc.tile_pool(name="x", bufs=2 * NCH8)),
            ctx.enter_context(tc.tile_pool(name="xb", bufs=8)),
            ctx.enter_context(tc.tile_pool(name="w", bufs=3 * NCH8 + 4)),
            ctx.enter_context(tc.tile_pool(name="wb", bufs=8)),
            ctx.enter_context(tc.tile_pool(name="wo", bufs=2)),
            ctx.enter_context(tc.tile_pool(name="wob", bufs=NP)),
            ctx.enter_context(tc.tile_pool(name="qk", bufs=2 * NP)),
            ctx.enter_context(tc.tile_pool(name="v", bufs=8)),
            ctx.enter_context(tc.tile_pool(name="vb", bufs=4)),
            ctx.enter_context(tc.tile_pool(name="sin", bufs=2)),
            ctx.enter_context(tc.tile_pool(name="pt", bufs=8)),
            ctx.enter_context(tc.tile_pool(name="ptb", bufs=8)),
            ctx.enter_context(tc.tile_pool(name="yt", bufs=2)),
            ctx.enter_context(tc.tile_pool(name="ytb", bufs=NP)),
            ctx.enter_context(tc.tile_pool(name="r", bufs=4)),
            ctx.enter_context(tc.tile_pool(name="os", bufs=4)),
            ctx.enter_context(tc.tile_pool(name="psp", bufs=2, space="PSUM")),
            ctx.enter_context(tc.tile_pool(name="pss", bufs=2, space="PSUM")),
            ctx.enter_context(tc.tile_pool(name="pso", bufs=2, space="PSUM")),
        )
        for rep in range(reps):
            _emit_body(nc, P, dram, rep)

    nc.finalize()
    _CACHED[key] = nc
    return nc


def _host_prep(x, wq, wk, wv, wo):
    """Fold RoPE rotation + fp8 scale into the weights; slice per core."""
    rope_dim = AOD // 2
    j = np.arange(rope_dim, dtype=np.float32)
    thetas = (1.0 / (10000.0 ** (2.0 * j / rope_dim))).astype(np.float32)
    pos = np.arange(S, dtype=np.float32)
    angles = pos[:, None] * thetas[None, :]          # [S, 32]
    sinv = np.sin(angles).astype(np.float32)         # [S, 32]
    # sin pattern tile [128, S]: row r multiplies feature (64*pair + r%64);
    # /16 descales the x128 fp8 weight scaling down to qt = 8*q
    sin2 = (np.tile(sinv.T, (4, 1)) / 16.0).astype(np.float32)

    def fold(w):
        wr = w.reshape(D, H, 2, rope_dim)
        return np.concatenate(
            [wr[:, :, 0] - wr[:, :, 1], wr[:, :, 0] + wr[:, :, 1]],
            axis=2).reshape(D, D)

    wqf = fold(wq) * 128.0
    wkf = fold(wk) * 128.0

    bf = ml_dtypes.bfloat16
    f8 = ml_dtypes.float8_e4m3

    def dr(w):  # [D, M] -> [NCH8, 128, 2, M] DoubleRow layout, fp8
        m = w.shape[1]
        return np.ascontiguousarray(
            w.reshape(NCH8, 2, 128, m).transpose(0, 2, 1, 3)).astype(f8)

    in_maps = []
    for core in range(NCORES):
        b, g = divmod(core, 2)
        sl = slice(g * FL, (g + 1) * FL)
        xT = np.ascontiguousarray(x[b].T)            # [D, S] f32
        wo_l = wo[sl, :] * 8.0                       # [FL, D]
        in_maps.append({
            "x8": dr(xT),
            "xb": np.ascontiguousarray(
                xT[:, 0:512].reshape(8, 128, 512)).astype(bf),
            "wq8": dr(wqf[:, sl]),
            "wk8": dr(wkf[:, sl]),
            "wv8": dr(wv[:, sl] * 128.0),
            "wvb": np.ascontiguousarray(
                (wv[:, sl] * 128.0).reshape(8, 128, FL)).astype(bf),
            "wo8": np.ascontiguousarray(
                wo_l.reshape(2, 2, 128, D).transpose(0, 2, 1, 3)).astype(f8),
            "wob": np.ascontiguousarray(
                wo_l.reshape(NP, 128, D)).astype(bf),
            "sinq": sin2.astype(bf),
        })
    return in_maps


def kernel(x, wq, wk, wv, wo, bo):
    nc = build_nc()
    in_maps = _host_prep(np.asarray(x, np.float32), np.asarray(wq, np.float32),
                         np.asarray(wk, np.float32), np.asarray(wv, np.float32),
                         np.asarray(wo, np.float32))
    res = run_bass_kernel_spmd(nc, in_maps, list(range(NCORES)))
    out = np.empty((B, S, D), np.float32)
    bo32 = np.asarray(bo, np.float32)
    for b in range(B):
        out[b] = res.results[2 * b]["out"] + res.results[2 * b + 1]["out"] + bo32
    return out
